# revision 21
# baseline (speedup 1.0000x reference)
"""Trainium2 Bass kernel for nn_LookaheadModel (topk_masking).

Sharding: data-parallel over batch B=8 (one batch element per core) for the
encoder; tiny AllGather of per-batch context vectors; vocab-sharded output
projection (each core computes logits[:, shard]).

v2 design:
- fp16 matmul path (embed/W1/W2/qW/out_W in fp16) -> FWL weight loads hide
  LDWEIGHTS behind matmul streaming.
- XBAR DMA transpose for the token->feature-major flip (no PE transposes).
- LayerNorm folding: phase A stores yT = (h0+ff)^T (pre-LN, fp16) plus
  per-token row stats S1=sum(y), S2=sum(y^2) and gate rows aR=(g.gw1)^T y,
  bR=(g.gw2)^T y. All LN effects applied later as per-token scalar algebra:
  h = (y - m) * r * g + b_ln  =>  w^T h = r*(gw)^T y - r*m*(w^T g) + w^T b_ln.
- Phase B runs in (128,32) column layout (partition p holds tokens
  [32p, 32p+32)), with tiny DRAM roundtrips to re-layout rows.
- Exact top-256 via 3-round 128-way threshold search + tie fix (as v1).
- out_W prefetched into SBUF at kernel start; AllGather of ctx as in v1.

Self-contained: only needs numpy + the system-installed concourse package.
"""

import numpy as np

import bass_rust
import concourse.bass as bass
import concourse.mybir as mybir
from concourse.bass_utils import run_bass_kernel_spmd
from concourse.tile import TileContext

AF = mybir.ActivationFunctionType
ALU = mybir.AluOpType
F32 = mybir.dt.float32
F32R = mybir.dt.float32r
F16 = mybir.dt.float16
I32 = mybir.dt.int32

# ---------------------------------------------------------------------------
# Workaround: this walrus build rejects any instruction carrying more than one
# sync-wait command. Hoist excess waits onto same-engine NOPs (sequential on
# the same engine queue, so semantically identical).
# ---------------------------------------------------------------------------
_MAX_WAITS = 1
_nop_counter = [0]


def _split_waits_in_ordered(nc, ordered):
    for bb_name, insts in ordered.items():
        out = []
        for inst in insts:
            si = inst.sync_info
            waits = list(si.on_wait) if si and si.on_wait else []
            if len(waits) > _MAX_WAITS:
                spill, keep = waits[:-_MAX_WAITS], waits[-_MAX_WAITS:]
                for i in range(0, len(spill), _MAX_WAITS):
                    _nop_counter[0] += 1
                    nop = bass_rust.InstNoOp(name=f"WSPILL-{_nop_counter[0]}")
                    nop.engine = inst.engine
                    nop.sync_info = mybir.SyncInfo(
                        on_wait=list(spill[i : i + _MAX_WAITS]), on_update=[]
                    )
                    nop.bass_nofuse = True
                    nc.register_instruction(nop, overwrite=True)
                    out.append(nop)
                si.on_wait = keep
            out.append(inst)
        if len(out) != len(insts):
            insts[:] = out


_orig_lower = TileContext._lower_ordered_insts
_orig_drain = TileContext._drain_and_barrier


def _lower_with_split(self, ordered):
    _split_waits_in_ordered(self.nc, ordered)
    return _orig_lower(self, ordered)


def _drain_and_barrier_split(self, tick_clock, wait_clock):
    nc = self.nc
    sc = bass_rust.ScopedClock({None: tick_clock.global_clock})
    drain_inst = nc.sync.drain()
    wait_clock.add_sem_waits(drain_inst.ins, sc)
    si = drain_inst.ins.sync_info
    waits = list(si.on_wait or [])
    if len(waits) > _MAX_WAITS:
        si.on_wait = waits[:_MAX_WAITS]
        rest = waits[_MAX_WAITS:]
        for i in range(0, len(rest), _MAX_WAITS):
            nop = nc.sync.nop(nofuse=True, hint=f"drain_wait_spill_{i}")
            nop.ins.sync_info = mybir.SyncInfo(
                on_wait=list(rest[i : i + _MAX_WAITS]), on_update=[]
            )
    nc.all_engine_barrier()
    popped = nc._tile_sem_poison_stack.pop()
    assert popped is self._sem_poison
    nc.clear_and_free_semaphores(list(self.sems.allocated().values()))
    nc.all_engine_barrier()


def _apply_patch():
    TileContext._drain_and_barrier = _drain_and_barrier_split
    TileContext._lower_ordered_insts = _lower_with_split


# ---------------------------------------------------------------------------
# Problem constants
# ---------------------------------------------------------------------------
V, D, SLOTS, K = 50257, 512, 256, 8
B, T = 8, 4096
NCORES = 8
VS = 6283  # vocab shard width per core; 8*6283 = 50264 >= V
NCH = 8  # T chunks of width 512
CW = 512
NK = D // 128  # 4 feature tiles
NF = 2 * D // 128  # 8 hidden tiles
BIG = 1.0e30
EPS = 1e-5
TPAD = T + 128  # padded row length for shifted window loads

PURE_FP32 = False  # kept for test.py compat; ignored (always fp16 path)
DEBUG_HT = False  # adds a (D, T) dump of yT per core (bring-up only)

_cache = {}


def build_bass():
    _apply_patch()
    nc = bass.Bass(trn_type="TRN2", num_devices=NCORES)

    # ---- I/O ----
    embed_h = nc.dram_tensor("embed_h", (V, D), F16, kind="ExternalInput")
    seq_idx = nc.dram_tensor("seq_idx", (128, 32), I32, kind="ExternalInput")
    w1h_d = nc.dram_tensor("w1h", (D, 2 * D), F16, kind="ExternalInput")
    w2h_d = nc.dram_tensor("w2h", (2 * D, D), F16, kind="ExternalInput")
    qwh_d = nc.dram_tensor("qwh", (D, D), F16, kind="ExternalInput")
    b1c = nc.dram_tensor("b1c", (128, NF), F32, kind="ExternalInput")
    b2c = nc.dram_tensor("b2c", (128, NK), F32, kind="ExternalInput")
    gcol_d = nc.dram_tensor("gcol", (128, NK), F32, kind="ExternalInput")
    bcol_d = nc.dram_tensor("bcol", (128, NK), F32, kind="ExternalInput")
    gw12_d = nc.dram_tensor("gw12", (128, 2 * NK), F16, kind="ExternalInput")
    qbc = nc.dram_tensor("qbc", (128, NK), F32, kind="ExternalInput")
    ident_in = nc.dram_tensor("ident", (128, 128), F32, kind="ExternalInput")
    onesc_in = nc.dram_tensor("onesc", (128, 1), F32, kind="ExternalInput")
    ones1x128_in = nc.dram_tensor("ones1x128", (1, 128), F32, kind="ExternalInput")
    ones1x8h_in = nc.dram_tensor("ones1x8h", (1, 8), F16, kind="ExternalInput")
    ones1x128h_in = nc.dram_tensor("ones1x128h", (1, 128), F16, kind="ExternalInput")
    alpha_in = nc.dram_tensor("alphac", (128, 1), F32, kind="ExternalInput")
    cnt32_in = nc.dram_tensor("cnt32", (128, 32), F32, kind="ExternalInput")
    # eps, gate_b, c1=gw1.g, c2=gw1.b, c3=gw2.g, c4=gw2.b, 0, 0
    sc_in = nc.dram_tensor("sc_in", (1, 8), F32, kind="ExternalInput")
    wout = nc.dram_tensor("wout", (D, VS), F16, kind="ExternalInput")
    bout = nc.dram_tensor("bout", (1, VS), F16, kind="ExternalInput")

    logits = nc.dram_tensor("logits", (B, VS), F32, kind="ExternalOutput")
    dbg = nc.dram_tensor("dbg", (5, T), F32, kind="ExternalOutput")
    if DEBUG_HT:
        ytdump = nc.dram_tensor("ytdump", (D, T), F32, kind="ExternalOutput")

    # DRAM scratch rows
    s1d = nc.dram_tensor("s1d", (1, T), F32, kind="Internal")
    s2d = nc.dram_tensor("s2d", (1, T), F32, kind="Internal")
    ard = nc.dram_tensor("ard", (1, T), F32, kind="Internal")
    brd = nc.dram_tensor("brd", (1, T), F32, kind="Internal")
    bfd = nc.dram_tensor("bfd", (1, TPAD), F32, kind="Internal")
    zd = nc.dram_tensor("zd", (1, T), F32, kind="Internal")
    sd = nc.dram_tensor("sd", (1, T), F32, kind="Internal")
    wd = nc.dram_tensor("wd", (1, T), F16, kind="Internal")

    cc_in = nc.dram_tensor("cc_in", (128, NK), F32, kind="Internal")
    cc_out = nc.dram_tensor(
        "cc_out", (128 * NCORES, NK), F32, kind="Internal", addr_space="Shared"
    )

    with TileContext(nc) as tc:
        with tc.tile_pool(name="consts", bufs=1) as cpool:
            # ---------------- persistent constants / weight prefetch -------
            # out_W first: big transfer, overlaps all of phase A.
            wot = []
            for k in range(NK):
                wt = cpool.tile([128, VS], F16, name=f"wot{k}")
                nc.sync.dma_start(wt[:], wout[128 * k : 128 * (k + 1), :])
                wot.append(wt)


            w1t = []
            for k in range(NK):
                wt = cpool.tile([128, 2 * D], F16, name=f"w1t{k}")
                nc.sync.dma_start(wt[:], w1h_d[128 * k : 128 * (k + 1), :])
                w1t.append(wt)
            w2t = []
            for k in range(NF):
                wt = cpool.tile([128, D], F16, name=f"w2t{k}")
                nc.sync.dma_start(wt[:], w2h_d[128 * k : 128 * (k + 1), :])
                w2t.append(wt)
            qwt = []
            for k in range(NK):
                wt = cpool.tile([128, D], F16, name=f"qwt{k}")
                nc.sync.dma_start(wt[:], qwh_d[128 * k : 128 * (k + 1), :])
                qwt.append(wt)

            ident = cpool.tile([128, 128], F32, name="ident_t")
            nc.sync.dma_start(ident[:], ident_in[:])
            b1t = cpool.tile([128, NF], F32, name="b1t")
            nc.sync.dma_start(b1t[:], b1c[:])
            b2t = cpool.tile([128, NK], F32, name="b2t")
            nc.sync.dma_start(b2t[:], b2c[:])
            gcol = cpool.tile([128, NK], F32, name="gcol_t")
            nc.sync.dma_start(gcol[:], gcol_d[:])
            bcol = cpool.tile([128, NK], F32, name="bcol_t")
            nc.sync.dma_start(bcol[:], bcol_d[:])
            gw12 = cpool.tile([128, 2 * NK], F16, name="gw12_t")
            nc.sync.dma_start(gw12[:], gw12_d[:])
            qbt = cpool.tile([128, NK], F32, name="qbt")
            nc.sync.dma_start(qbt[:], qbc[:])
            onescol = cpool.tile([128, 1], F32, name="onescol")
            nc.sync.dma_start(onescol[:], onesc_in[:])
            onescol_r = cpool.tile([128, 1], F32R, name="onescol_r")
            nc.vector.tensor_copy(onescol_r[:], onescol[:])
            onescol_h = cpool.tile([128, 1], F16, name="onescol_h")
            nc.vector.tensor_copy(onescol_h[:], onescol[:])
            ones1x128 = cpool.tile([1, 128], F32, name="ones1x128")
            nc.sync.dma_start(ones1x128[:], ones1x128_in[:])
            ones1x128h = cpool.tile([1, 128], F16, name="ones1x128h")
            nc.sync.dma_start(ones1x128h[:], ones1x128h_in[:])
            ones1x8h = cpool.tile([1, 8], F16, name="ones1x8h")
            nc.sync.dma_start(ones1x8h[:], ones1x8h_in[:])
            alphac = cpool.tile([128, 1], F32, name="alphac_t")
            nc.sync.dma_start(alphac[:], alpha_in[:])
            cnt32 = cpool.tile([128, 32], F32, name="cnt32_t")
            nc.sync.dma_start(cnt32[:], cnt32_in[:])
            scin = cpool.tile([1, 8], F32, name="scin")
            nc.sync.dma_start(scin[:], sc_in[:])
            eps_ap = scin[0:1, 0:1]
            gb_ap = scin[0:1, 1:2]
            c1_ap = scin[0:1, 2:3]
            c2_ap = scin[0:1, 3:4]
            c3_ap = scin[0:1, 4:5]
            c4_ap = scin[0:1, 5:6]
            sidx = cpool.tile([128, 32], I32, name="sidx")
            nc.sync.dma_start(sidx[:], seq_idx[:])
            zpad = cpool.tile([1, 128], F32, name="zpad")
            nc.vector.memset(zpad[:], 0.0)
            nc.sync.dma_start(bfd[0:1, T:TPAD], zpad[:])

            strip = cpool.tile([1, 64], F32, name="strip")
            ctx4 = cpool.tile([128, NK], F32, name="ctx4")
            ctxall = cpool.tile([128, 32], F32, name="ctxall")

            with tc.tile_pool(name="yT", bufs=1) as ypool_p:
                yT = [ypool_p.tile([128, T], F16, name=f"yT{k}") for k in range(NK)]

                # ---------------- phase A: gather + FFN (pre-LN) ----------
                with (
                    tc.tile_pool(name="gat", bufs=3) as gpool,
                    tc.tile_pool(name="txp", bufs=4) as txpool,
                    tc.tile_pool(name="x0p", bufs=2) as x0pool,
                    tc.tile_pool(name="ap", bufs=2) as apool,
                    tc.tile_pool(name="sqp", bufs=2) as sqpool,
                    tc.tile_pool(name="strp", bufs=2) as strpool,
                    tc.tile_pool(name="psa", bufs=2, space="PSUM") as psa,
                    tc.tile_pool(name="psf", bufs=2, space="PSUM") as psf,
                    tc.tile_pool(name="psr", bufs=1, space="PSUM") as psr,
                ):
                    for ch in range(NCH):
                        sl = slice(ch * CW, (ch + 1) * CW)
                        x0 = [
                            x0pool.tile([128, CW], F16, name=f"x0_{k}", tag=f"x0_{k}")
                            for k in range(NK)
                        ]
                        for blk in range(4):
                            tb = 4 * ch + blk
                            g = gpool.tile([128, D], F16, name="g", tag="g")
                            nc.gpsimd.indirect_dma_start(
                                out=g[:],
                                out_offset=None,
                                in_=embed_h[:],
                                in_offset=bass.IndirectOffsetOnAxis(
                                    ap=sidx[:, tb : tb + 1], axis=0
                                ),
                            )
                            for k in range(NK):
                                tx = txpool.tile(
                                    [128, 128], F16, name="tx", tag=f"tx{k}"
                                )
                                nc.sync.dma_start_transpose(
                                    tx[:], g[:, 128 * k : 128 * (k + 1)]
                                )
                                nc.sync.dma_start(
                                    x0[k][:, 128 * blk : 128 * (blk + 1)], tx[:]
                                )
                        # layer 1 + relu
                        af = [
                            apool.tile([128, CW], F16, name=f"af{m}", tag=f"af{m}")
                            for m in range(NF)
                        ]
                        for m in range(NF):
                            ps = psa.tile([128, CW], F32, tag="psa")
                            for k in range(NK):
                                nc.tensor.matmul(
                                    ps[:],
                                    lhsT=w1t[k][:, 128 * m : 128 * (m + 1)],
                                    rhs=x0[k][:],
                                    start=(k == 0),
                                    stop=(k == NK - 1),
                                )
                            nc.scalar.activation(
                                af[m][:], ps[:], AF.Relu, bias=b1t[:, m : m + 1]
                            )
                        # layer 2 + bias + residual -> yT (fp16)
                        yc_ps = []
                        for m in range(NK):
                            ps = psf.tile([128, CW], F32, tag="psf")
                            for k in range(NF):
                                nc.tensor.matmul(
                                    ps[:],
                                    lhsT=w2t[k][:, 128 * m : 128 * (m + 1)],
                                    rhs=af[k][:],
                                    start=(k == 0),
                                    stop=(k == NF - 1),
                                )
                            nc.vector.scalar_tensor_tensor(
                                out=yT[m][:, sl],
                                in0=ps[:],
                                scalar=b2t[:, m : m + 1],
                                in1=x0[m][:],
                                op0=ALU.add,
                                op1=ALU.add,
                            )
                        # row stats + gate rows (each its own (1,CW) psum bank)
                        rp_s1 = psr.tile([1, CW], F32, tag="rs1")
                        rp_s2 = psr.tile([1, CW], F32, tag="rs2")
                        rp_ar = psr.tile([1, CW], F32, tag="rar")
                        rp_br = psr.tile([1, CW], F32, tag="rbr")
                        for m in range(NK):
                            nc.tensor.matmul(
                                rp_s1[:],
                                lhsT=onescol_h[:],
                                rhs=yT[m][:, sl],
                                start=(m == 0),
                                stop=(m == NK - 1),
                            )
                        for m in range(NK):
                            sq = sqpool.tile([128, CW], F16, name="sq", tag="sq")
                            nc.scalar.activation(sq[:], yT[m][:, sl], AF.Square)
                            nc.tensor.matmul(
                                rp_s2[:],
                                lhsT=onescol_h[:],
                                rhs=sq[:],
                                start=(m == 0),
                                stop=(m == NK - 1),
                            )
                        for m in range(NK):
                            nc.tensor.matmul(
                                rp_ar[:],
                                lhsT=gw12[:, m : m + 1],
                                rhs=yT[m][:, sl],
                                start=(m == 0),
                                stop=(m == NK - 1),
                            )
                        for m in range(NK):
                            nc.tensor.matmul(
                                rp_br[:],
                                lhsT=gw12[:, NK + m : NK + m + 1],
                                rhs=yT[m][:, sl],
                                start=(m == 0),
                                stop=(m == NK - 1),
                            )
                        strp = strpool.tile([1, 4 * CW], F32, name="strp", tag="strp")
                        nc.vector.tensor_copy(strp[0:1, 0:CW], rp_s1[:])
                        nc.vector.tensor_copy(strp[0:1, CW : 2 * CW], rp_s2[:])
                        nc.scalar.activation(strp[0:1, 2 * CW : 3 * CW], rp_ar[:], AF.Copy)
                        nc.scalar.activation(strp[0:1, 3 * CW : 4 * CW], rp_br[:], AF.Copy)
                        nc.sync.dma_start(s1d[0:1, sl], strp[0:1, 0:CW])
                        nc.sync.dma_start(s2d[0:1, sl], strp[0:1, CW : 2 * CW])
                        nc.sync.dma_start(ard[0:1, sl], strp[0:1, 2 * CW : 3 * CW])
                        nc.sync.dma_start(brd[0:1, sl], strp[0:1, 3 * CW : 4 * CW])

                    if DEBUG_HT:
                        for k in range(NK):
                            ydf = sqpool.tile([128, T], F32, name=f"ydf{k}", tag="ydf")
                            nc.vector.tensor_copy(ydf[:], yT[k][:])
                            nc.sync.dma_start(
                                ytdump[128 * k : 128 * (k + 1), :], ydf[:]
                            )

                # ---------------- phase B ------------------------------
                with (
                    tc.tile_pool(name="colp", bufs=1) as colp,
                    tc.tile_pool(name="shp", bufs=1) as shp,
                    tc.tile_pool(name="rowp", bufs=2) as rowp,
                    tc.tile_pool(name="bigp", bufs=1) as bigp,
                    tc.tile_pool(name="pssm", bufs=1, space="PSUM") as ps_small,
                    tc.tile_pool(name="psrow", bufs=2, space="PSUM") as ps_row,
                    tc.tile_pool(name="pswd", bufs=2, space="PSUM") as ps_wide,
                ):
                    # -- small helpers ------------------------------------
                    def pe_bcast_col(src11, dst_col):
                        # broadcast a (1,1) scalar to a (128,1) column
                        p = ps_small.tile([128, 1], F32, tag="bc")
                        nc.tensor.matmul(
                            p[:], lhsT=ones1x128[:], rhs=src11, start=True, stop=True
                        )
                        nc.vector.tensor_copy(dst_col, p[:])

                    def col_reduce(src_col, dst11, op):
                        # reduce a (128,1) column to (1,1): transpose + reduce
                        p = ps_small.tile([1, 128], F32, tag="tr")
                        nc.tensor.transpose(p[:], src_col, ident[:])
                        if op == "max":
                            nc.vector.reduce_max(
                                out=dst11, in_=p[:], axis=mybir.AxisListType.X
                            )
                        elif op == "min":
                            nc.vector.tensor_reduce(
                                out=dst11, in_=p[:], axis=mybir.AxisListType.X,
                                op=ALU.min,
                            )
                        else:
                            nc.vector.tensor_reduce(
                                out=dst11, in_=p[:], axis=mybir.AxisListType.X,
                                op=ALU.add,
                            )

                    # -- q vector (exact h at t=T-1) ----------------------
                    # m,r for last token from S1/S2 scalars
                    s1l = colp.tile([1, 2], F32, name="s1l")
                    nc.sync.dma_start(s1l[0:1, 0:1], s1d[0:1, T - 1 : T])
                    nc.sync.dma_start(s1l[0:1, 1:2], s2d[0:1, T - 1 : T])
                    ml_ap = strip[0:1, 0:1]
                    nc.vector.tensor_scalar(
                        out=ml_ap, in0=s1l[0:1, 0:1], scalar1=1.0 / D, scalar2=None,
                        op0=ALU.mult,
                    )
                    e2l = strip[0:1, 1:2]
                    nc.vector.tensor_scalar(
                        out=e2l, in0=s1l[0:1, 1:2], scalar1=1.0 / D, scalar2=None,
                        op0=ALU.mult,
                    )
                    varl = strip[0:1, 2:3]
                    tmpl = strip[0:1, 3:4]
                    nc.vector.tensor_mul(tmpl, ml_ap, ml_ap)
                    nc.vector.tensor_sub(varl, e2l, tmpl)
                    nc.scalar.activation(tmpl, varl, AF.Ln, bias=eps_ap)
                    rl_ap = strip[0:1, 4:5]
                    nc.scalar.activation(rl_ap, tmpl, AF.Exp, scale=-0.5)

                    mlc = colp.tile([128, 1], F32, name="mlc")
                    rlc = colp.tile([128, 1], F32, name="rlc")
                    pe_bcast_col(ml_ap, mlc[:])
                    pe_bcast_col(rl_ap, rlc[:])

                    ylast = colp.tile([128, NK], F32, name="ylast")
                    for k in range(NK):
                        nc.vector.tensor_copy(
                            ylast[:, k : k + 1], yT[k][:, T - 1 : T]
                        )
                    hlast = colp.tile([128, NK], F32, name="hlast")
                    # h = (y - m) * r * g + b
                    nc.vector.tensor_scalar(
                        out=hlast[:], in0=ylast[:], scalar1=mlc[:, 0:1],
                        scalar2=None, op0=ALU.subtract,
                    )
                    nc.vector.tensor_scalar(
                        out=hlast[:], in0=hlast[:], scalar1=rlc[:, 0:1],
                        scalar2=None, op0=ALU.mult,
                    )
                    nc.vector.tensor_mul(hlast[:], hlast[:], gcol[:])
                    nc.vector.tensor_add(hlast[:], hlast[:], bcol[:])
                    hlast_h = colp.tile([128, NK], F16, name="hlast_h")
                    nc.vector.tensor_copy(hlast_h[:], hlast[:])

                    qh = colp.tile([128, NK], F32, name="qh")
                    for j in range(NK):
                        pq = ps_small.tile([128, 1], F32, tag="pq")
                        for k in range(NK):
                            nc.tensor.matmul(
                                pq[:],
                                lhsT=qwt[k][:, 128 * j : 128 * (j + 1)],
                                rhs=hlast_h[:, k : k + 1],
                                start=(k == 0),
                                stop=(k == NK - 1),
                            )
                        nc.vector.tensor_add(qh[:, j : j + 1], pq[:], qbt[:, j : j + 1])
                    # qg = q * g (for score matmuls); c5 = q.g ; c6 = q.b
                    qg = colp.tile([128, NK], F32, name="qg")
                    nc.vector.tensor_mul(qg[:], qh[:], gcol[:])
                    qgh = colp.tile([128, NK], F16, name="qgh")
                    nc.vector.tensor_copy(qgh[:], qg[:])
                    dotc = colp.tile([128, 1], F32, name="dotc")
                    scr4 = colp.tile([128, NK], F32, name="scr4")
                    nc.vector.scalar_tensor_tensor(
                        out=scr4[:], in0=qh[:], scalar=1.0, in1=gcol[:],
                        op0=ALU.mult, op1=ALU.mult, accum_out=dotc[:, 0:1],
                    )
                    c5_ap = strip[0:1, 5:6]
                    col_reduce(dotc[:, 0:1], c5_ap, "sum")
                    nc.vector.scalar_tensor_tensor(
                        out=scr4[:], in0=qh[:], scalar=1.0, in1=bcol[:],
                        op0=ALU.mult, op1=ALU.mult, accum_out=dotc[:, 0:1],
                    )
                    c6_ap = strip[0:1, 6:7]
                    col_reduce(dotc[:, 0:1], c6_ap, "sum")

                    # -- raw score row: sR = (qg)^T y, per chunk ----------
                    for ch in range(NCH):
                        sl = slice(ch * CW, (ch + 1) * CW)
                        pss = ps_row.tile([1, CW], F32, tag="pss")
                        for k in range(NK):
                            nc.tensor.matmul(
                                pss[:],
                                lhsT=qgh[:, k : k + 1],
                                rhs=yT[k][:, sl],
                                start=(k == 0),
                                stop=(k == NK - 1),
                            )
                        sev = rowp.tile([1, CW], F32, name="sev", tag="sev")
                        nc.scalar.activation(sev[:], pss[:], AF.Copy)
                        nc.sync.dma_start(sd[0:1, sl], sev[:])

                    # -- per-token stats in (128,32) layout ---------------
                    c32 = lambda nm: colp.tile([128, 32], F32, name=nm)
                    s1c = c32("s1c")
                    nc.sync.dma_start(
                        s1c[:], s1d[:].rearrange("o (p c) -> (o p) c", p=128)
                    )
                    s2c = c32("s2c")
                    nc.sync.dma_start(
                        s2c[:], s2d[:].rearrange("o (p c) -> (o p) c", p=128)
                    )
                    arc = c32("arc")
                    nc.sync.dma_start(
                        arc[:], ard[:].rearrange("o (p c) -> (o p) c", p=128)
                    )
                    brc = c32("brc")
                    nc.sync.dma_start(
                        brc[:], brd[:].rearrange("o (p c) -> (o p) c", p=128)
                    )
                    m32 = c32("m32")
                    nc.vector.tensor_scalar(
                        out=m32[:], in0=s1c[:], scalar1=1.0 / D, scalar2=None,
                        op0=ALU.mult,
                    )
                    var32 = c32("var32")
                    nc.vector.tensor_scalar(
                        out=var32[:], in0=s2c[:], scalar1=1.0 / D, scalar2=None,
                        op0=ALU.mult,
                    )
                    t32 = c32("t32")
                    nc.vector.tensor_mul(t32[:], m32[:], m32[:])
                    nc.vector.tensor_sub(var32[:], var32[:], t32[:])
                    epscol = colp.tile([128, 1], F32, name="epscol")
                    pe_bcast_col(eps_ap, epscol[:])
                    nc.vector.tensor_scalar(
                        out=t32[:], in0=var32[:], scalar1=epscol[:, 0:1],
                        scalar2=None, op0=ALU.add,
                    )
                    ln32 = c32("ln32")
                    nc.scalar.activation(ln32[:], t32[:], AF.Ln)
                    r32 = c32("r32")
                    nc.scalar.activation(r32[:], ln32[:], AF.Exp, scale=-0.5)
                    rm32 = c32("rm32")
                    nc.vector.tensor_mul(rm32[:], r32[:], m32[:])

                    # gate rows -> a32, b32:  x = r*xR - rm*cG + cB
                    ccols = colp.tile([128, 4], F32, name="ccols")
                    pe_bcast_col(c1_ap, ccols[:, 0:1])
                    pe_bcast_col(c2_ap, ccols[:, 1:2])
                    pe_bcast_col(c3_ap, ccols[:, 2:3])
                    pe_bcast_col(c4_ap, ccols[:, 3:4])

                    def ln_row_fix(out32, raw32, cg, cb):
                        nc.vector.tensor_mul(out32[:], raw32[:], r32[:])
                        nc.vector.tensor_scalar(
                            out=t32[:], in0=rm32[:], scalar1=cg, scalar2=None,
                            op0=ALU.mult,
                        )
                        nc.vector.tensor_sub(out32[:], out32[:], t32[:])
                        nc.vector.tensor_scalar(
                            out=out32[:], in0=out32[:], scalar1=cb, scalar2=None,
                            op0=ALU.add,
                        )

                    a32 = c32("a32")
                    ln_row_fix(a32, arc, ccols[:, 0:1], ccols[:, 1:2])
                    b32 = c32("b32")
                    ln_row_fix(b32, brc, ccols[:, 2:3], ccols[:, 3:4])

                    # windowed lookahead sum: u_t = sum_{s=1..8} b_{t+s}
                    nc.sync.dma_start(
                        bfd[0:1, 0:T].rearrange("o (p c) -> (o p) c", p=128), b32[:]
                    )
                    bsh = []
                    for s in range(1, K + 1):
                        bt_ = shp.tile([128, 32], F32, name=f"bsh{s}", tag=f"bsh{s}")
                        nc.sync.dma_start(
                            bt_[:],
                            bfd[0:1, s : s + T].rearrange(
                                "o (p c) -> (o p) c", p=128
                            ),
                        )
                        bsh.append(bt_)
                    u32 = c32("u32")
                    nc.vector.tensor_add(u32[:], bsh[0][:], bsh[1][:])
                    for s in range(2, K):
                        nc.vector.tensor_add(u32[:], u32[:], bsh[s][:])
                    # z = a + gate_b + u * cntrec
                    z32 = c32("z32")
                    nc.vector.tensor_mul(t32[:], u32[:], cnt32[:])
                    nc.vector.tensor_add(z32[:], a32[:], t32[:])
                    gbcol = colp.tile([128, 1], F32, name="gbcol")
                    pe_bcast_col(gb_ap, gbcol[:])
                    nc.vector.tensor_scalar(
                        out=z32[:], in0=z32[:], scalar1=gbcol[:, 0:1],
                        scalar2=None, op0=ALU.add,
                    )
                    nc.sync.dma_start(
                        dbg[0:1, :].rearrange("o (p c) -> (o p) c", p=128), z32[:]
                    )
                    nc.sync.dma_start(
                        zd[0:1, 0:T].rearrange("o (p c) -> (o p) c", p=128), z32[:]
                    )

                    # -- selection: 3-round 128-way threshold search ------
                    zB = bigp.tile([128, T], F32, name="zB")
                    scrB = bigp.tile([128, T], mybir.dt.uint8, name="scrB")
                    for ch in range(NCH):
                        sl = slice(ch * CW, (ch + 1) * CW)
                        zrc = rowp.tile([1, CW], F32, name="zrc", tag="zrc")
                        nc.sync.dma_start(zrc[:], zd[0:1, sl])
                        pb = ps_wide.tile([128, CW], F32, tag="pb")
                        nc.tensor.matmul(
                            pb[:], lhsT=ones1x128[:], rhs=zrc[:],
                            start=True, stop=True,
                        )
                        if ch % 2 == 0:
                            nc.vector.tensor_copy(zB[:, sl], pb[:])
                        else:
                            nc.scalar.activation(zB[:, sl], pb[:], AF.Copy)

                    coltmp = colp.tile([128, 16], F32, name="coltmp")
                    mn_c = coltmp[:, 0:1]
                    mx_c = coltmp[:, 1:2]
                    nc.vector.tensor_reduce(
                        out=mn_c, in_=z32[:], axis=mybir.AxisListType.X, op=ALU.min
                    )
                    nc.vector.reduce_max(out=mx_c, in_=z32[:], axis=mybir.AxisListType.X)
                    lo0 = strip[0:1, 8:9]
                    hi0 = strip[0:1, 9:10]
                    col_reduce(mn_c, lo0, "min")
                    col_reduce(mx_c, hi0, "max")

                    N_ROUNDS = 3
                    lo_cur, hi_cur = lo0, hi0
                    si = 10
                    tau_col = coltmp[:, 2:3]
                    dB = coltmp[:, 3:4]
                    loB = coltmp[:, 4:5]
                    cnt_col = coltmp[:, 5:6]
                    sgn_col = coltmp[:, 6:7]
                    for r in range(N_ROUNDS):
                        d0 = strip[0:1, si : si + 1]
                        nc.vector.tensor_sub(d0, hi_cur, lo_cur)
                        pe_bcast_col(d0, dB)
                        pe_bcast_col(lo_cur, loB)
                        nc.vector.tensor_mul(tau_col, alphac[:], dB)
                        nc.vector.tensor_add(tau_col, tau_col, loB)
                        nc.vector.scalar_tensor_tensor(
                            out=scrB[:],
                            in0=zB[:],
                            scalar=tau_col,
                            in1=zB[:],
                            op0=ALU.is_gt,
                            op1=ALU.bypass,
                            accum_out=cnt_col,
                        )
                        nc.vector.tensor_scalar(
                            out=sgn_col, in0=cnt_col, scalar1=float(SLOTS),
                            scalar2=None, op0=ALU.is_ge,
                        )
                        pj = ps_small.tile([1, 1], F32, tag="pj")
                        nc.tensor.matmul(
                            pj[:], lhsT=sgn_col, rhs=onescol[:], start=True, stop=True
                        )
                        dd = strip[0:1, si + 1 : si + 2]
                        nc.vector.tensor_scalar(
                            out=dd, in0=d0, scalar1=1.0 / 128, scalar2=None,
                            op0=ALU.mult,
                        )
                        tmp = strip[0:1, si + 2 : si + 3]
                        nc.vector.tensor_mul(tmp, pj[:], dd)
                        lo_n = strip[0:1, si + 3 : si + 4]
                        nc.vector.tensor_add(lo_n, lo_cur, tmp)
                        hi_n = strip[0:1, si + 4 : si + 5]
                        nc.vector.tensor_add(hi_n, lo_n, dd)
                        lo_cur, hi_cur = lo_n, hi_n
                        si += 5

                    # v0 = min(z > lo_cur) exactly, from (128,32) cols
                    loB2 = coltmp[:, 7:8]
                    pe_bcast_col(lo_cur, loB2)
                    mask_u8 = colp.tile([128, 32], mybir.dt.uint8, name="mask_u8")
                    nc.vector.tensor_scalar(
                        out=mask_u8[:], in0=z32[:], scalar1=loB2, scalar2=None,
                        op0=ALU.is_gt,
                    )
                    w_c = colp.tile([128, 32], F32, name="w_c")
                    nc.vector.memset(w_c[:], BIG)
                    nc.vector.copy_predicated(w_c[:], mask_u8[:], z32[:])
                    wmin_c = coltmp[:, 8:9]
                    nc.vector.tensor_reduce(
                        out=wmin_c, in_=w_c[:], axis=mybir.AxisListType.X, op=ALU.min
                    )
                    v0 = strip[0:1, si : si + 1]
                    col_reduce(wmin_c, v0, "min")
                    # c2cnt = count(z > v0); need = 256 - c2cnt
                    vB = coltmp[:, 9:10]
                    pe_bcast_col(v0, vB)
                    gt32 = c32("gt32")
                    nc.vector.scalar_tensor_tensor(
                        out=gt32[:], in0=z32[:], scalar=vB, in1=z32[:],
                        op0=ALU.is_gt, op1=ALU.bypass, accum_out=coltmp[:, 10:11],
                    )
                    pc2 = ps_small.tile([1, 1], F32, tag="pj")
                    nc.tensor.matmul(
                        pc2[:], lhsT=coltmp[:, 10:11], rhs=onescol[:],
                        start=True, stop=True,
                    )
                    need0 = strip[0:1, si + 1 : si + 2]
                    nc.vector.tensor_scalar(
                        out=need0, in0=pc2[:], scalar1=float(SLOTS), scalar2=-1.0,
                        op0=ALU.subtract, op1=ALU.mult,
                    )
                    needcol = coltmp[:, 11:12]
                    pe_bcast_col(need0, needcol)

                    # tie-fix: prefix count of (z == v0) in T order
                    maskeq = c32("maskeq")
                    nc.vector.tensor_scalar(
                        out=maskeq[:], in0=z32[:], scalar1=vB, scalar2=None,
                        op0=ALU.is_equal,
                    )
                    iscan = c32("iscan")
                    nc.vector.tensor_tensor_scan(
                        iscan[:], maskeq[:], maskeq[:], 0.0,
                        op0=ALU.add, op1=ALU.bypass,
                    )
                    # exclusive partition offsets: scan of per-partition totals
                    ptot_tr = ps_small.tile([1, 128], F32, tag="tr")
                    nc.tensor.transpose(ptot_tr[:], iscan[:, 31:32], ident[:])
                    ptr_row = rowp.tile([1, 300], F32, name="ptr_row", tag="ptr_row")
                    nc.vector.tensor_copy(ptr_row[0:1, 0:128], ptot_tr[:])
                    nc.vector.tensor_tensor_scan(
                        ptr_row[0:1, 128:256], ptr_row[0:1, 0:128],
                        ptr_row[0:1, 0:128], 0.0, op0=ALU.add, op1=ALU.bypass,
                    )
                    nc.vector.tensor_sub(
                        ptr_row[0:1, 128:256], ptr_row[0:1, 128:256],
                        ptr_row[0:1, 0:128],
                    )
                    offs_ps = ps_small.tile([128, 1], F32, tag="bc")
                    nc.tensor.matmul(
                        offs_ps[:], lhsT=ptr_row[0:1, 128:256],
                        rhs=ones1x128[0:1, 0:1],
                        start=True, stop=True,
                    )
                    offcol = coltmp[:, 12:13]
                    nc.vector.tensor_copy(offcol, offs_ps[:])
                    incl = c32("incl")
                    nc.vector.tensor_scalar(
                        out=incl[:], in0=iscan[:], scalar1=offcol, scalar2=None,
                        op0=ALU.add,
                    )
                    fill = c32("fill")
                    nc.vector.scalar_tensor_tensor(
                        out=fill[:], in0=incl[:], scalar=needcol, in1=maskeq[:],
                        op0=ALU.is_le, op1=ALU.mult,
                    )
                    sel32 = c32("sel32")
                    nc.vector.tensor_scalar(
                        out=sel32[:], in0=z32[:], scalar1=vB, scalar2=None,
                        op0=ALU.is_gt,
                    )
                    nc.vector.tensor_add(sel32[:], sel32[:], fill[:])
                    nc.sync.dma_start(
                        dbg[1:2, :].rearrange("o (p c) -> (o p) c", p=128), sel32[:]
                    )

                    # -- masked softmax in (128,32) -----------------------
                    src = c32("src")
                    nc.sync.dma_start(
                        src[:], sd[:].rearrange("o (p c) -> (o p) c", p=128)
                    )
                    s32 = c32("s32")
                    c56 = colp.tile([128, 2], F32, name="c56")
                    pe_bcast_col(c5_ap, c56[:, 0:1])
                    pe_bcast_col(c6_ap, c56[:, 1:2])
                    ln_row_fix(s32, src, c56[:, 0:1], c56[:, 1:2])
                    nc.sync.dma_start(
                        dbg[2:3, :].rearrange("o (p c) -> (o p) c", p=128), s32[:]
                    )
                    masked = c32("masked")
                    nc.vector.tensor_scalar(
                        out=masked[:], in0=sel32[:], scalar1=-1.0, scalar2=BIG,
                        op0=ALU.add, op1=ALU.mult,
                    )
                    nc.vector.tensor_add(masked[:], masked[:], s32[:])
                    nc.vector.reduce_max(
                        out=coltmp[:, 13:14], in_=masked[:], axis=mybir.AxisListType.X
                    )
                    smax = strip[0:1, si + 2 : si + 3]
                    col_reduce(coltmp[:, 13:14], smax, "max")
                    nsmax = strip[0:1, si + 3 : si + 4]
                    nc.vector.tensor_scalar(
                        out=nsmax, in0=smax, scalar1=-1.0, scalar2=None, op0=ALU.mult
                    )
                    nsmaxcol = coltmp[:, 14:15]
                    pe_bcast_col(nsmax, nsmaxcol)
                    e32h = colp.tile([128, 32], F16, name="e32h")
                    zsumcol = colp.tile([128, 1], F32, name="zsumcol")
                    nc.scalar.activation(
                        e32h[:], masked[:], AF.Exp, bias=nsmaxcol,
                        accum_out=zsumcol[:, 0:1],
                    )
                    zsum = strip[0:1, si + 4 : si + 5]
                    col_reduce(zsumcol[:, 0:1], zsum, "sum")
                    rz = strip[0:1, si + 5 : si + 6]
                    nc.vector.reciprocal(out=rz, in_=zsum)
                    # ctx weights w = e * r ; sigma = sum(w * m)
                    w32h = colp.tile([128, 32], F16, name="w32h")
                    nc.vector.tensor_mul(w32h[:], e32h[:], r32[:])
                    sig32 = c32("sig32")
                    nc.vector.scalar_tensor_tensor(
                        out=sig32[:], in0=m32[:], scalar=1.0, in1=w32h[:],
                        op0=ALU.mult, op1=ALU.mult, accum_out=coltmp[:, 15:16],
                    )
                    sig = strip[0:1, si + 6 : si + 7]
                    col_reduce(coltmp[:, 15:16], sig, "sum")
                    nc.sync.dma_start(
                        wd[0:1, 0:T].rearrange("o (p c) -> (o p) c", p=128), w32h[:]
                    )

                    # broadcast w row and accumulate ctxraw over yT
                    wB = bigp.tile([128, T], F16, name="wB")
                    for ch in range(NCH):
                        sl = slice(ch * CW, (ch + 1) * CW)
                        wrc = rowp.tile([1, CW], F16, name="wrc", tag="wrc")
                        nc.sync.dma_start(wrc[:], wd[0:1, sl])
                        pb = ps_wide.tile([128, CW], F32, tag="pb")
                        nc.tensor.matmul(
                            pb[:], lhsT=ones1x128h[:], rhs=wrc[:],
                            start=True, stop=True,
                        )
                        if ch % 2 == 0:
                            nc.vector.tensor_copy(wB[:, sl], pb[:])
                        else:
                            nc.scalar.activation(wB[:, sl], pb[:], AF.Copy)
                    scrH = bigp.tile([128, T], F16, name="scrH")
                    for k in range(NK):
                        nc.vector.scalar_tensor_tensor(
                            out=scrH[:],
                            in0=yT[k][:],
                            scalar=1.0,
                            in1=wB[:],
                            op0=ALU.mult,
                            op1=ALU.mult,
                            accum_out=ctx4[:, k : k + 1],
                        )
                    # ctx = (g*ctxraw - sig*g) * rz + b
                    sigcol = coltmp[:, 13:14]
                    pe_bcast_col(sig, sigcol)
                    rzcol = coltmp[:, 14:15]
                    pe_bcast_col(rz, rzcol)
                    nc.vector.tensor_scalar(
                        out=ctx4[:], in0=ctx4[:], scalar1=sigcol, scalar2=None,
                        op0=ALU.subtract,
                    )
                    nc.vector.tensor_mul(ctx4[:], ctx4[:], gcol[:])
                    nc.vector.tensor_scalar(
                        out=ctx4[:], in0=ctx4[:], scalar1=rzcol, scalar2=None,
                        op0=ALU.mult,
                    )
                    nc.vector.tensor_add(ctx4[:], ctx4[:], bcol[:])
                # yT/phase-B pools closed

            # ---------------- allgather + output projection ----------------
            nc.sync.dma_start(cc_in[:], ctx4[:])
            nc.gpsimd.collective_compute(
                "AllGather",
                ALU.bypass,
                replica_groups=[list(range(NCORES))],
                ins=[cc_in[:]],
                outs=[cc_out[:]],
            )
            nc.sync.dma_start(
                ctxall[:].rearrange("p (j b) -> p j b", j=NK),
                cc_out[:].rearrange("(b p) j -> p j b", p=128),
            )
            ctxall_h = cpool.tile([128, 32], F16, name="ctxall_h")
            nc.vector.tensor_copy(ctxall_h[:], ctxall[:])
            nchunks = (VS + CW - 1) // CW
            with (
                tc.tile_pool(name="lo", bufs=4) as lopool,
                tc.tile_pool(name="psl", bufs=4, space="PSUM") as psl,
            ):
                for n in range(nchunks):
                    w = min(CW, VS - n * CW)
                    vsl = slice(n * CW, n * CW + w)
                    bt = lopool.tile([1, CW], F16, name="bo", tag="bo")
                    nc.sync.dma_start(bt[:, :w], bout[:, vsl])
                    pl = psl.tile([B, CW], F32, tag="pl")
                    for k in range(NK):
                        nc.tensor.matmul(
                            pl[:, :w],
                            lhsT=ctxall_h[:, 8 * k : 8 * (k + 1)],
                            rhs=wot[k][:, vsl],
                            start=(k == 0),
                            stop=False,
                        )
                    nc.tensor.matmul(
                        pl[:, :w], lhsT=ones1x8h[:], rhs=bt[:, :w],
                        start=False, stop=True,
                    )
                    lt = lopool.tile([B, CW], F32, name="lt", tag="lt")
                    nc.vector.tensor_copy(lt[:, :w], pl[:, :w])
                    nc.sync.dma_start(logits[:, vsl], lt[:, :w])

    return nc


def _host_prep(inputs):
    f32 = lambda a: np.ascontiguousarray(np.asarray(a, dtype=np.float32))
    f16c = lambda a: np.ascontiguousarray(np.asarray(a, dtype=np.float16))
    seq = np.asarray(inputs["seq"])
    embed = f32(inputs["embed"])
    w1 = f32(inputs["W1"])
    b1 = f32(inputs["b1"])
    w2 = f32(inputs["W2"])
    b2 = f32(inputs["b2"])
    ln_g = f32(inputs["ln_g"])
    ln_b = f32(inputs["ln_b"])
    gw = f32(inputs["gate_W"])
    gb = f32(inputs["gate_b"])
    qw = f32(inputs["q_W"])
    qb = f32(inputs["q_b"])
    wout_f = f32(inputs["out_W"])
    bout_f = f32(inputs["out_b"])

    colpack = lambda v: np.ascontiguousarray(
        v.reshape(-1, 128).T.astype(np.float32)
    )  # (Ntiles*128,) -> (128, Ntiles); tile k col = dims [128k, 128k+128)
    cnt = np.minimum(K, T - 1 - np.arange(T)).astype(np.float32)
    cntrec = np.zeros(T, dtype=np.float32)
    cntrec[cnt > 0] = 1.0 / cnt[cnt > 0]

    gw1 = gw[:D, 0]
    gw2 = gw[D:, 0]
    gw1g = (gw1 * ln_g).astype(np.float32)
    gw2g = (gw2 * ln_g).astype(np.float32)
    gw12 = np.concatenate(
        [colpack(gw1g), colpack(gw2g)], axis=1
    ).astype(np.float16)
    c1 = float(np.dot(gw1, ln_g))
    c2 = float(np.dot(gw1, ln_b))
    c3 = float(np.dot(gw2, ln_g))
    c4 = float(np.dot(gw2, ln_b))

    base = {
        "embed_h": f16c(embed),
        "w1h": f16c(w1),
        "w2h": f16c(w2),
        "qwh": f16c(qw),
        "b1c": colpack(b1),
        "b2c": colpack(b2),
        "gcol": colpack(ln_g),
        "bcol": colpack(ln_b),
        "gw12": np.ascontiguousarray(gw12),
        "qbc": colpack(qb),
        "ident": np.eye(128, dtype=np.float32),
        "onesc": np.ones((128, 1), dtype=np.float32),
        "ones1x128": np.ones((1, 128), dtype=np.float32),
        "ones1x128h": np.ones((1, 128), dtype=np.float16),
        "ones1x8h": np.ones((1, 8), dtype=np.float16),
        "alphac": ((np.arange(128, dtype=np.float32) + 1.0) / 128.0).reshape(128, 1),
        "cnt32": np.ascontiguousarray(cntrec.reshape(128, 32)),
        "sc_in": np.array(
            [[EPS, float(gb[0]), c1, c2, c3, c4, 0.0, 0.0]], dtype=np.float32
        ),
    }
    wout_pad = np.zeros((D, NCORES * VS), dtype=np.float32)
    wout_pad[:, :V] = wout_f
    bout_pad = np.zeros(NCORES * VS, dtype=np.float32)
    bout_pad[:V] = bout_f

    in_maps = []
    for c in range(NCORES):
        m = dict(base)
        m["seq_idx"] = np.ascontiguousarray(
            seq[c].reshape(32, 128).T.astype(np.int32)
        )
        m["wout"] = np.ascontiguousarray(
            wout_pad[:, c * VS : (c + 1) * VS].astype(np.float16)
        )
        m["bout"] = np.ascontiguousarray(
            bout_pad[c * VS : (c + 1) * VS].reshape(1, VS).astype(np.float16)
        )
        in_maps.append(m)
    return in_maps


def get_nc():
    key = (DEBUG_HT,)
    if key not in _cache:
        _cache[key] = build_bass()
    return _cache[key]


def run_full(inputs, trace=False):
    """Run the kernel; returns (logits_full, BassKernelResults)."""
    nc = get_nc()
    in_maps = _host_prep(inputs)
    res = run_bass_kernel_spmd(
        nc, in_maps, core_ids=list(range(NCORES)), trace=trace
    )
    parts = [res.results[c]["logits"] for c in range(NCORES)]
    logits = np.concatenate(parts, axis=1)[:, :V]
    return logits, res


def kernel(**inputs) -> np.ndarray:
    logits, _ = run_full(inputs, trace=False)
    return logits


# revision 29
# speedup vs baseline: 1.5769x; 1.5769x over previous
"""Trainium2 Bass kernel for nn_LookaheadModel (topk_masking).

Sharding: data-parallel over batch B=8 (one batch element per core) for the
encoder; tiny AllGather of per-batch context vectors; vocab-sharded output
projection (each core computes logits[:, shard]).

v2 design:
- fp16 matmul path (embed/W1/W2/qW/out_W in fp16) -> FWL weight loads hide
  LDWEIGHTS behind matmul streaming.
- XBAR DMA transpose for the token->feature-major flip (no PE transposes).
- LayerNorm folding: phase A stores yT = (h0+ff)^T (pre-LN, fp16) plus
  per-token row stats S1=sum(y), S2=sum(y^2) and gate rows aR=(g.gw1)^T y,
  bR=(g.gw2)^T y. All LN effects applied later as per-token scalar algebra:
  h = (y - m) * r * g + b_ln  =>  w^T h = r*(gw)^T y - r*m*(w^T g) + w^T b_ln.
- Phase B runs in (128,32) column layout (partition p holds tokens
  [32p, 32p+32)), with tiny DRAM roundtrips to re-layout rows.
- Exact top-256 via 3-round 128-way threshold search + tie fix (as v1).
- out_W prefetched into SBUF at kernel start; AllGather of ctx as in v1.

Self-contained: only needs numpy + the system-installed concourse package.
"""

import numpy as np

import bass_rust
import concourse.bass as bass
import concourse.mybir as mybir
from concourse.bass_utils import run_bass_kernel_spmd
from concourse.tile import TileContext

AF = mybir.ActivationFunctionType
ALU = mybir.AluOpType
F32 = mybir.dt.float32
F32R = mybir.dt.float32r
F16 = mybir.dt.float16
I32 = mybir.dt.int32

# ---------------------------------------------------------------------------
# Workaround: this walrus build rejects any instruction carrying more than one
# sync-wait command. Hoist excess waits onto same-engine NOPs (sequential on
# the same engine queue, so semantically identical).
# ---------------------------------------------------------------------------
_MAX_WAITS = 1
_nop_counter = [0]


def _split_waits_in_ordered(nc, ordered):
    for bb_name, insts in ordered.items():
        out = []
        for inst in insts:
            si = inst.sync_info
            waits = list(si.on_wait) if si and si.on_wait else []
            if len(waits) > _MAX_WAITS:
                spill, keep = waits[:-_MAX_WAITS], waits[-_MAX_WAITS:]
                for i in range(0, len(spill), _MAX_WAITS):
                    _nop_counter[0] += 1
                    nop = bass_rust.InstNoOp(name=f"WSPILL-{_nop_counter[0]}")
                    nop.engine = inst.engine
                    nop.sync_info = mybir.SyncInfo(
                        on_wait=list(spill[i : i + _MAX_WAITS]), on_update=[]
                    )
                    nop.bass_nofuse = True
                    nc.register_instruction(nop, overwrite=True)
                    out.append(nop)
                si.on_wait = keep
            out.append(inst)
        if len(out) != len(insts):
            insts[:] = out


_orig_lower = TileContext._lower_ordered_insts
_orig_drain = TileContext._drain_and_barrier


def _lower_with_split(self, ordered):
    _split_waits_in_ordered(self.nc, ordered)
    return _orig_lower(self, ordered)


def _drain_and_barrier_split(self, tick_clock, wait_clock):
    nc = self.nc
    sc = bass_rust.ScopedClock({None: tick_clock.global_clock})
    drain_inst = nc.sync.drain()
    wait_clock.add_sem_waits(drain_inst.ins, sc)
    si = drain_inst.ins.sync_info
    waits = list(si.on_wait or [])
    if len(waits) > _MAX_WAITS:
        si.on_wait = waits[:_MAX_WAITS]
        rest = waits[_MAX_WAITS:]
        for i in range(0, len(rest), _MAX_WAITS):
            nop = nc.sync.nop(nofuse=True, hint=f"drain_wait_spill_{i}")
            nop.ins.sync_info = mybir.SyncInfo(
                on_wait=list(rest[i : i + _MAX_WAITS]), on_update=[]
            )
    nc.all_engine_barrier()
    popped = nc._tile_sem_poison_stack.pop()
    assert popped is self._sem_poison
    nc.clear_and_free_semaphores(list(self.sems.allocated().values()))
    nc.all_engine_barrier()


def _apply_patch():
    TileContext._drain_and_barrier = _drain_and_barrier_split
    TileContext._lower_ordered_insts = _lower_with_split


# ---------------------------------------------------------------------------
# Problem constants
# ---------------------------------------------------------------------------
V, D, SLOTS, K = 50257, 512, 256, 8
B, T = 8, 4096
NCORES = 8
VS = 6283  # vocab shard width per core; 8*6283 = 50264 >= V
NCH = 8  # T chunks of width 512
CW = 512
NK = D // 128  # 4 feature tiles
NF = 2 * D // 128  # 8 hidden tiles
BIG = 1.0e30
EPS = 1e-5
TPAD = T + 128  # padded row length for shifted window loads

PURE_FP32 = False  # kept for test.py compat; ignored (always fp16 path)
DEBUG_HT = False  # adds a (D, T) dump of yT per core (bring-up only)

_cache = {}


def build_bass():
    _apply_patch()
    nc = bass.Bass(trn_type="TRN2", num_devices=NCORES)

    # ---- I/O ----
    embed_h = nc.dram_tensor("embed_h", (V, D), F16, kind="ExternalInput")
    seq_idx = nc.dram_tensor("seq_idx", (128, 32), I32, kind="ExternalInput")
    w1h_d = nc.dram_tensor("w1h", (D, 2 * D), F16, kind="ExternalInput")
    w2h_d = nc.dram_tensor("w2h", (2 * D, D), F16, kind="ExternalInput")
    qwh_d = nc.dram_tensor("qwh", (D, D), F16, kind="ExternalInput")
    b1c = nc.dram_tensor("b1c", (128, NF), F32, kind="ExternalInput")
    b2c = nc.dram_tensor("b2c", (128, NK), F32, kind="ExternalInput")
    gcol_d = nc.dram_tensor("gcol", (128, NK), F32, kind="ExternalInput")
    bcol_d = nc.dram_tensor("bcol", (128, NK), F32, kind="ExternalInput")
    gw12_d = nc.dram_tensor("gw12", (128, 2 * NK), F16, kind="ExternalInput")
    qbc = nc.dram_tensor("qbc", (128, NK), F32, kind="ExternalInput")
    ident_in = nc.dram_tensor("ident", (128, 128), F32, kind="ExternalInput")
    onesc_in = nc.dram_tensor("onesc", (128, 1), F32, kind="ExternalInput")
    ones1x128_in = nc.dram_tensor("ones1x128", (1, 128), F32, kind="ExternalInput")
    ones1x8h_in = nc.dram_tensor("ones1x8h", (1, 8), F16, kind="ExternalInput")
    ones1x128h_in = nc.dram_tensor("ones1x128h", (1, 128), F16, kind="ExternalInput")
    alpha_in = nc.dram_tensor("alphac", (128, 1), F32, kind="ExternalInput")
    cnt32_in = nc.dram_tensor("cnt32", (128, 32), F32, kind="ExternalInput")
    # eps, gate_b, c1=gw1.g, c2=gw1.b, c3=gw2.g, c4=gw2.b, 0, 0
    sc_in = nc.dram_tensor("sc_in", (1, 8), F32, kind="ExternalInput")
    wout = nc.dram_tensor("wout", (D, VS), F16, kind="ExternalInput")
    bout = nc.dram_tensor("bout", (1, VS), F16, kind="ExternalInput")

    logits = nc.dram_tensor("logits", (B, VS), F32, kind="ExternalOutput")
    dbg = nc.dram_tensor("dbg", (5, T), F32, kind="ExternalOutput")
    if DEBUG_HT:
        ytdump = nc.dram_tensor("ytdump", (D, T), F32, kind="ExternalOutput")

    # DRAM scratch rows
    s1d = nc.dram_tensor("s1d", (1, T), F32, kind="Internal")
    s2d = nc.dram_tensor("s2d", (1, T), F32, kind="Internal")
    ard = nc.dram_tensor("ard", (1, T), F32, kind="Internal")
    brd = nc.dram_tensor("brd", (1, T), F32, kind="Internal")
    bfd = nc.dram_tensor("bfd", (1, TPAD), F32, kind="Internal")
    zd = nc.dram_tensor("zd", (1, T), F32, kind="Internal")
    sd = nc.dram_tensor("sd", (1, T), F32, kind="Internal")
    wd = nc.dram_tensor("wd", (1, T), F16, kind="Internal")

    cc_in = nc.dram_tensor("cc_in", (128, NK), F32, kind="Internal")
    cc_out = nc.dram_tensor(
        "cc_out", (128 * NCORES, NK), F32, kind="Internal", addr_space="Shared"
    )

    with TileContext(nc) as tc:
        with tc.tile_pool(name="consts", bufs=1) as cpool:
            # ---------------- persistent constants / weight prefetch -------
            # out_W on the scalar HWDGE queue: big transfer, overlaps phase A
            # without blocking the sync queue.
            wot = []
            for k in range(NK):
                wt = cpool.tile([128, VS], F16, name=f"wot{k}")
                nc.scalar.dma_start(wt[:], wout[128 * k : 128 * (k + 1), :])
                wot.append(wt)


            w1t = []
            for k in range(NK):
                wt = cpool.tile([128, 2 * D], F16, name=f"w1t{k}")
                nc.sync.dma_start(wt[:], w1h_d[128 * k : 128 * (k + 1), :])
                w1t.append(wt)
            w2t = []
            for k in range(NF):
                wt = cpool.tile([128, D], F16, name=f"w2t{k}")
                nc.sync.dma_start(wt[:], w2h_d[128 * k : 128 * (k + 1), :])
                w2t.append(wt)
            qwt = []
            for k in range(NK):
                wt = cpool.tile([128, D], F16, name=f"qwt{k}")
                nc.sync.dma_start(wt[:], qwh_d[128 * k : 128 * (k + 1), :])
                qwt.append(wt)

            ident = cpool.tile([128, 128], F32, name="ident_t")
            nc.sync.dma_start(ident[:], ident_in[:])
            ident_h = cpool.tile([128, 128], F16, name="ident_h")
            nc.vector.tensor_copy(ident_h[:], ident[:])
            b1t = cpool.tile([128, NF], F32, name="b1t")
            nc.sync.dma_start(b1t[:], b1c[:])
            b2t = cpool.tile([128, NK], F32, name="b2t")
            nc.sync.dma_start(b2t[:], b2c[:])
            gcol = cpool.tile([128, NK], F32, name="gcol_t")
            nc.sync.dma_start(gcol[:], gcol_d[:])
            bcol = cpool.tile([128, NK], F32, name="bcol_t")
            nc.sync.dma_start(bcol[:], bcol_d[:])
            gw12 = cpool.tile([128, 2 * NK], F16, name="gw12_t")
            nc.sync.dma_start(gw12[:], gw12_d[:])
            qbt = cpool.tile([128, NK], F32, name="qbt")
            nc.sync.dma_start(qbt[:], qbc[:])
            onescol = cpool.tile([128, 1], F32, name="onescol")
            nc.sync.dma_start(onescol[:], onesc_in[:])
            onescol_r = cpool.tile([128, 1], F32R, name="onescol_r")
            nc.vector.tensor_copy(onescol_r[:], onescol[:])
            onescol_h = cpool.tile([128, 1], F16, name="onescol_h")
            nc.vector.tensor_copy(onescol_h[:], onescol[:])
            ones1x128 = cpool.tile([1, 128], F32, name="ones1x128")
            nc.sync.dma_start(ones1x128[:], ones1x128_in[:])
            ones1x128h = cpool.tile([1, 128], F16, name="ones1x128h")
            nc.sync.dma_start(ones1x128h[:], ones1x128h_in[:])
            ones1x8h = cpool.tile([1, 8], F16, name="ones1x8h")
            nc.sync.dma_start(ones1x8h[:], ones1x8h_in[:])
            alphac = cpool.tile([128, 1], F32, name="alphac_t")
            nc.sync.dma_start(alphac[:], alpha_in[:])
            cnt32 = cpool.tile([128, 32], F32, name="cnt32_t")
            nc.sync.dma_start(cnt32[:], cnt32_in[:])
            scin = cpool.tile([1, 8], F32, name="scin")
            nc.sync.dma_start(scin[:], sc_in[:])
            eps_ap = scin[0:1, 0:1]
            gb_ap = scin[0:1, 1:2]
            c1_ap = scin[0:1, 2:3]
            c2_ap = scin[0:1, 3:4]
            c3_ap = scin[0:1, 4:5]
            c4_ap = scin[0:1, 5:6]
            sidx = cpool.tile([128, 32], I32, name="sidx")
            nc.sync.dma_start(sidx[:], seq_idx[:])
            zpad = cpool.tile([1, 128], F32, name="zpad")
            nc.vector.memset(zpad[:], 0.0)
            nc.sync.dma_start(bfd[0:1, T:TPAD], zpad[:])

            strip = cpool.tile([1, 64], F32, name="strip")
            ctx4 = cpool.tile([128, NK], F32, name="ctx4")
            ctxall = cpool.tile([128, 32], F32, name="ctxall")

            with tc.tile_pool(name="yT", bufs=1) as ypool_p:
                yT = [ypool_p.tile([128, T], F16, name=f"yT{k}") for k in range(NK)]

                # ---------------- phase A: gather + FFN (pre-LN) ----------
                with (
                    tc.tile_pool(name="gat", bufs=3) as gpool,
                    tc.tile_pool(name="x0p", bufs=2) as x0pool,
                    tc.tile_pool(name="ap", bufs=2) as apool,
                    tc.tile_pool(name="sqp", bufs=2) as sqpool,
                    tc.tile_pool(name="strp", bufs=2) as strpool,
                    tc.tile_pool(name="pstp", bufs=2, space="PSUM") as pstp,
                    tc.tile_pool(name="psa", bufs=2, space="PSUM") as psa,
                    tc.tile_pool(name="psf", bufs=2, space="PSUM") as psf,
                    tc.tile_pool(name="psr", bufs=1, space="PSUM") as psr,
                ):
                    # software-pipelined: rows(ch-1) interleaves with L1(ch)
                    def do_rows_a(ch):
                        # S1 = ones^T y ; S2 = ones^T y^2 (sq from ACT)
                        sl = slice(ch * CW, (ch + 1) * CW)
                        rp_s1 = psr.tile([1, CW], F32, tag="rs1")
                        rp_s2 = psr.tile([1, CW], F32, tag="rs2")
                        for m in range(NK):
                            nc.tensor.matmul(
                                rp_s1[:],
                                lhsT=onescol_h[:],
                                rhs=yT[m][:, sl],
                                start=(m == 0),
                                stop=(m == NK - 1),
                            )
                        for m in range(NK):
                            sq = sqpool.tile([128, CW], F16, name="sq", tag="sq")
                            nc.scalar.activation(sq[:], yT[m][:, sl], AF.Square)
                            nc.tensor.matmul(
                                rp_s2[:],
                                lhsT=onescol_h[:],
                                rhs=sq[:],
                                start=(m == 0),
                                stop=(m == NK - 1),
                            )
                        strp = strpool.tile(
                            [1, 4 * CW], F32, name="strp", tag="strp"
                        )
                        nc.vector.tensor_copy(strp[0:1, 0:CW], rp_s1[:])
                        nc.vector.tensor_copy(strp[0:1, CW : 2 * CW], rp_s2[:])
                        nc.sync.dma_start(s1d[0:1, sl], strp[0:1, 0:CW])
                        nc.sync.dma_start(s2d[0:1, sl], strp[0:1, CW : 2 * CW])
                        return strp

                    def do_rows_b(ch, strp):
                        sl = slice(ch * CW, (ch + 1) * CW)
                        rp_ar = psr.tile([1, CW], F32, tag="rs1")
                        rp_br = psr.tile([1, CW], F32, tag="rs2")
                        for m in range(NK):
                            nc.tensor.matmul(
                                rp_ar[:],
                                lhsT=gw12[:, m : m + 1],
                                rhs=yT[m][:, sl],
                                start=(m == 0),
                                stop=(m == NK - 1),
                            )
                        for m in range(NK):
                            nc.tensor.matmul(
                                rp_br[:],
                                lhsT=gw12[:, NK + m : NK + m + 1],
                                rhs=yT[m][:, sl],
                                start=(m == 0),
                                stop=(m == NK - 1),
                            )
                        nc.vector.tensor_copy(strp[0:1, 2 * CW : 3 * CW], rp_ar[:])
                        nc.vector.tensor_copy(strp[0:1, 3 * CW : 4 * CW], rp_br[:])
                        nc.sync.dma_start(ard[0:1, sl], strp[0:1, 2 * CW : 3 * CW])
                        nc.sync.dma_start(brd[0:1, sl], strp[0:1, 3 * CW : 4 * CW])

                    prev = None  # (ch, strp) pending row-stats for prev chunk
                    for ch in range(NCH):
                        sl = slice(ch * CW, (ch + 1) * CW)
                        x0 = [
                            x0pool.tile([128, CW], F16, name=f"x0_{k}", tag=f"x0_{k}")
                            for k in range(NK)
                        ]
                        # gather + PE transpose (fp16, 1 cycle/row)
                        for blk in range(4):
                            tb = 4 * ch + blk
                            g = gpool.tile([128, D], F16, name="g", tag="g")
                            nc.gpsimd.indirect_dma_start(
                                out=g[:],
                                out_offset=None,
                                in_=embed_h[:],
                                in_offset=bass.IndirectOffsetOnAxis(
                                    ap=sidx[:, tb : tb + 1], axis=0
                                ),
                            )
                            tp = pstp.tile([128, D], F16, tag="tp")
                            for k in range(NK):
                                nc.tensor.transpose(
                                    tp[:, 128 * k : 128 * (k + 1)],
                                    g[:, 128 * k : 128 * (k + 1)],
                                    ident_h[:],
                                )
                            for k in range(NK):
                                dst = x0[k][:, 128 * blk : 128 * (blk + 1)]
                                src = tp[:, 128 * k : 128 * (k + 1)]
                                if k % 2 == 0:
                                    nc.vector.tensor_copy(dst, src)
                                else:
                                    nc.scalar.activation(dst, src, AF.Copy)
                        if prev is not None:
                            prev_strp = do_rows_a(prev)
                        # layer 1 + relu
                        af = [
                            apool.tile([128, CW], F16, name=f"af{m}", tag=f"af{m}")
                            for m in range(NF)
                        ]
                        for m in range(NF):
                            ps = psa.tile([128, CW], F32, tag="psa")
                            for k in range(NK):
                                nc.tensor.matmul(
                                    ps[:],
                                    lhsT=w1t[k][:, 128 * m : 128 * (m + 1)],
                                    rhs=x0[k][:],
                                    start=(k == 0),
                                    stop=(k == NK - 1),
                                )
                            nc.scalar.activation(
                                af[m][:], ps[:], AF.Relu, bias=b1t[:, m : m + 1]
                            )
                            if m == 3 and prev is not None:
                                do_rows_b(prev, prev_strp)
                                prev = None
                        # layer 2 + bias + residual -> yT (fp16)
                        for m in range(NK):
                            ps = psf.tile([128, CW], F32, tag="psf")
                            for k in range(NF):
                                nc.tensor.matmul(
                                    ps[:],
                                    lhsT=w2t[k][:, 128 * m : 128 * (m + 1)],
                                    rhs=af[k][:],
                                    start=(k == 0),
                                    stop=(k == NF - 1),
                                )
                            nc.vector.scalar_tensor_tensor(
                                out=yT[m][:, sl],
                                in0=ps[:],
                                scalar=b2t[:, m : m + 1],
                                in1=x0[m][:],
                                op0=ALU.add,
                                op1=ALU.add,
                            )
                        prev = ch
                    # drain last chunk's rows
                    last_strp = do_rows_a(NCH - 1)
                    do_rows_b(NCH - 1, last_strp)

                    if DEBUG_HT:
                        for k in range(NK):
                            ydf = sqpool.tile([128, T], F32, name=f"ydf{k}", tag="ydf")
                            nc.vector.tensor_copy(ydf[:], yT[k][:])
                            nc.sync.dma_start(
                                ytdump[128 * k : 128 * (k + 1), :], ydf[:]
                            )

                # ---------------- phase B ------------------------------
                with (
                    tc.tile_pool(name="colp", bufs=1) as colp,
                    tc.tile_pool(name="shp", bufs=1) as shp,
                    tc.tile_pool(name="rowp", bufs=2) as rowp,
                    tc.tile_pool(name="bigp", bufs=1) as bigp,
                    tc.tile_pool(name="pssm", bufs=1, space="PSUM") as ps_small,
                    tc.tile_pool(name="psrow", bufs=2, space="PSUM") as ps_row,
                    tc.tile_pool(name="pswd", bufs=2, space="PSUM") as ps_wide,
                ):
                    # -- small helpers ------------------------------------
                    def pe_bcast_col(src1x, dst_cols):
                        # broadcast a (1,n) row to (128,n) columns
                        n = src1x.shape[-1]
                        p = ps_small.tile([128, 8], F32, tag="bc")
                        nc.tensor.matmul(
                            p[:, :n], lhsT=ones1x128[:], rhs=src1x,
                            start=True, stop=True,
                        )
                        nc.vector.tensor_copy(dst_cols, p[:, :n])

                    # static scalars -> columns in one broadcast:
                    # 0=eps 1=gate_b 2=c1 3=c2 4=c3 5=c4
                    ccols8 = colp.tile([128, 8], F32, name="ccols8")
                    pe_bcast_col(scin[0:1, 0:8], ccols8[:])

                    def col_reduce(src_col, dst11, op):
                        # reduce a (128,1) column to (1,1): transpose + reduce
                        p = ps_small.tile([1, 128], F32, tag="tr")
                        nc.tensor.transpose(p[:], src_col, ident[:])
                        if op == "max":
                            nc.vector.reduce_max(
                                out=dst11, in_=p[:], axis=mybir.AxisListType.X
                            )
                        elif op == "min":
                            nc.vector.tensor_reduce(
                                out=dst11, in_=p[:], axis=mybir.AxisListType.X,
                                op=ALU.min,
                            )
                        else:
                            nc.vector.tensor_reduce(
                                out=dst11, in_=p[:], axis=mybir.AxisListType.X,
                                op=ALU.add,
                            )

                    # -- q vector (exact h at t=T-1) ----------------------
                    # m,r for last token from S1/S2 scalars
                    s1l = colp.tile([1, 2], F32, name="s1l")
                    nc.sync.dma_start(s1l[0:1, 0:1], s1d[0:1, T - 1 : T])
                    nc.sync.dma_start(s1l[0:1, 1:2], s2d[0:1, T - 1 : T])
                    ml_ap = strip[0:1, 0:1]
                    nc.vector.tensor_scalar(
                        out=ml_ap, in0=s1l[0:1, 0:1], scalar1=1.0 / D, scalar2=None,
                        op0=ALU.mult,
                    )
                    e2l = strip[0:1, 2:3]
                    nc.vector.tensor_scalar(
                        out=e2l, in0=s1l[0:1, 1:2], scalar1=1.0 / D, scalar2=None,
                        op0=ALU.mult,
                    )
                    varl = strip[0:1, 3:4]
                    tmpl = strip[0:1, 4:5]
                    nc.vector.tensor_mul(tmpl, ml_ap, ml_ap)
                    nc.vector.tensor_sub(varl, e2l, tmpl)
                    nc.scalar.activation(tmpl, varl, AF.Ln, bias=eps_ap)
                    rl_ap = strip[0:1, 1:2]
                    nc.scalar.activation(rl_ap, tmpl, AF.Exp, scale=-0.5)

                    mrlc = colp.tile([128, 2], F32, name="mrlc")
                    pe_bcast_col(strip[0:1, 0:2], mrlc[:])
                    mlc = mrlc[:, 0:1]
                    rlc = mrlc[:, 1:2]

                    ylast = colp.tile([128, NK], F32, name="ylast")
                    for k in range(NK):
                        nc.vector.tensor_copy(
                            ylast[:, k : k + 1], yT[k][:, T - 1 : T]
                        )
                    hlast = colp.tile([128, NK], F32, name="hlast")
                    # h = (y - m) * r * g + b
                    nc.vector.tensor_scalar(
                        out=hlast[:], in0=ylast[:], scalar1=mlc,
                        scalar2=None, op0=ALU.subtract,
                    )
                    nc.vector.tensor_scalar(
                        out=hlast[:], in0=hlast[:], scalar1=rlc,
                        scalar2=None, op0=ALU.mult,
                    )
                    nc.vector.tensor_mul(hlast[:], hlast[:], gcol[:])
                    nc.vector.tensor_add(hlast[:], hlast[:], bcol[:])
                    hlast_h = colp.tile([128, NK], F16, name="hlast_h")
                    nc.vector.tensor_copy(hlast_h[:], hlast[:])

                    qh = colp.tile([128, NK], F32, name="qh")
                    for j in range(NK):
                        pq = ps_small.tile([128, 1], F32, tag="pq")
                        for k in range(NK):
                            nc.tensor.matmul(
                                pq[:],
                                lhsT=qwt[k][:, 128 * j : 128 * (j + 1)],
                                rhs=hlast_h[:, k : k + 1],
                                start=(k == 0),
                                stop=(k == NK - 1),
                            )
                        nc.vector.tensor_add(qh[:, j : j + 1], pq[:], qbt[:, j : j + 1])
                    # qg = q * g (for score matmuls); c5 = q.g ; c6 = q.b
                    qg = colp.tile([128, NK], F32, name="qg")
                    nc.vector.tensor_mul(qg[:], qh[:], gcol[:])
                    qgh = colp.tile([128, NK], F16, name="qgh")
                    nc.vector.tensor_copy(qgh[:], qg[:])
                    dotc = colp.tile([128, 1], F32, name="dotc")
                    scr4 = colp.tile([128, NK], F32, name="scr4")
                    nc.vector.scalar_tensor_tensor(
                        out=scr4[:], in0=qh[:], scalar=1.0, in1=gcol[:],
                        op0=ALU.mult, op1=ALU.mult, accum_out=dotc[:, 0:1],
                    )
                    c5_ap = strip[0:1, 5:6]
                    col_reduce(dotc[:, 0:1], c5_ap, "sum")
                    nc.vector.scalar_tensor_tensor(
                        out=scr4[:], in0=qh[:], scalar=1.0, in1=bcol[:],
                        op0=ALU.mult, op1=ALU.mult, accum_out=dotc[:, 0:1],
                    )
                    c6_ap = strip[0:1, 6:7]
                    col_reduce(dotc[:, 0:1], c6_ap, "sum")

                    # -- raw score row: sR = (qg)^T y, per chunk ----------
                    for ch in range(NCH):
                        sl = slice(ch * CW, (ch + 1) * CW)
                        pss = ps_row.tile([1, CW], F32, tag="pss")
                        for k in range(NK):
                            nc.tensor.matmul(
                                pss[:],
                                lhsT=qgh[:, k : k + 1],
                                rhs=yT[k][:, sl],
                                start=(k == 0),
                                stop=(k == NK - 1),
                            )
                        sev = rowp.tile([1, CW], F32, name="sev", tag="sev")
                        nc.scalar.activation(sev[:], pss[:], AF.Copy)
                        nc.sync.dma_start(sd[0:1, sl], sev[:])

                    # -- per-token stats in (128,32) layout ---------------
                    c32 = lambda nm: colp.tile([128, 32], F32, name=nm)
                    s1c = c32("s1c")
                    nc.scalar.dma_start(
                        s1c[:], s1d[:].rearrange("o (p c) -> (o p) c", p=128)
                    )
                    s2c = c32("s2c")
                    nc.scalar.dma_start(
                        s2c[:], s2d[:].rearrange("o (p c) -> (o p) c", p=128)
                    )
                    arc = c32("arc")
                    nc.scalar.dma_start(
                        arc[:], ard[:].rearrange("o (p c) -> (o p) c", p=128)
                    )
                    brc = c32("brc")
                    nc.scalar.dma_start(
                        brc[:], brd[:].rearrange("o (p c) -> (o p) c", p=128)
                    )
                    m32 = c32("m32")
                    nc.vector.tensor_scalar(
                        out=m32[:], in0=s1c[:], scalar1=1.0 / D, scalar2=None,
                        op0=ALU.mult,
                    )
                    var32 = c32("var32")
                    nc.vector.tensor_scalar(
                        out=var32[:], in0=s2c[:], scalar1=1.0 / D, scalar2=None,
                        op0=ALU.mult,
                    )
                    t32 = c32("t32")
                    nc.vector.tensor_mul(t32[:], m32[:], m32[:])
                    nc.vector.tensor_sub(var32[:], var32[:], t32[:])
                    nc.vector.tensor_scalar(
                        out=t32[:], in0=var32[:], scalar1=ccols8[:, 0:1],
                        scalar2=None, op0=ALU.add,
                    )
                    ln32 = c32("ln32")
                    nc.scalar.activation(ln32[:], t32[:], AF.Ln)
                    r32 = c32("r32")
                    nc.scalar.activation(r32[:], ln32[:], AF.Exp, scale=-0.5)
                    rm32 = c32("rm32")
                    nc.vector.tensor_mul(rm32[:], r32[:], m32[:])

                    # gate rows -> a32, b32:  x = r*xR - rm*cG + cB

                    def ln_row_fix(out32, raw32, cg, cb):
                        nc.vector.tensor_mul(out32[:], raw32[:], r32[:])
                        nc.vector.tensor_scalar(
                            out=t32[:], in0=rm32[:], scalar1=cg, scalar2=None,
                            op0=ALU.mult,
                        )
                        nc.vector.tensor_sub(out32[:], out32[:], t32[:])
                        nc.vector.tensor_scalar(
                            out=out32[:], in0=out32[:], scalar1=cb, scalar2=None,
                            op0=ALU.add,
                        )

                    a32 = c32("a32")
                    ln_row_fix(a32, arc, ccols8[:, 2:3], ccols8[:, 3:4])
                    b32 = c32("b32")
                    ln_row_fix(b32, brc, ccols8[:, 4:5], ccols8[:, 5:6])

                    # windowed lookahead sum: u_t = sum_{s=1..8} b_{t+s}
                    nc.sync.dma_start(
                        bfd[0:1, 0:T].rearrange("o (p c) -> (o p) c", p=128), b32[:]
                    )
                    bsh = []
                    for s in range(1, K + 1):
                        bt_ = shp.tile([128, 32], F32, name=f"bsh{s}", tag=f"bsh{s}")
                        nc.scalar.dma_start(
                            bt_[:],
                            bfd[0:1, s : s + T].rearrange(
                                "o (p c) -> (o p) c", p=128
                            ),
                        )
                        bsh.append(bt_)
                    u32 = c32("u32")
                    nc.vector.tensor_add(u32[:], bsh[0][:], bsh[1][:])
                    for s in range(2, K):
                        nc.vector.tensor_add(u32[:], u32[:], bsh[s][:])
                    # z = a + gate_b + u * cntrec
                    z32 = c32("z32")
                    nc.vector.tensor_mul(t32[:], u32[:], cnt32[:])
                    nc.vector.tensor_add(z32[:], a32[:], t32[:])
                    nc.vector.tensor_scalar(
                        out=z32[:], in0=z32[:], scalar1=ccols8[:, 1:2],
                        scalar2=None, op0=ALU.add,
                    )
                    nc.sync.dma_start(
                        dbg[0:1, :].rearrange("o (p c) -> (o p) c", p=128), z32[:]
                    )
                    nc.sync.dma_start(
                        zd[0:1, 0:T].rearrange("o (p c) -> (o p) c", p=128), z32[:]
                    )

                    # -- selection: 3-round 128-way threshold search ------
                    zB = bigp.tile([128, T], F32, name="zB")
                    scrB = bigp.tile([128, T], mybir.dt.uint8, name="scrB")
                    for ch in range(NCH):
                        sl = slice(ch * CW, (ch + 1) * CW)
                        zrc = rowp.tile([1, CW], F32, name="zrc", tag="zrc")
                        nc.sync.dma_start(zrc[:], zd[0:1, sl])
                        pb = ps_wide.tile([128, CW], F32, tag="pb")
                        nc.tensor.matmul(
                            pb[:], lhsT=ones1x128[:], rhs=zrc[:],
                            start=True, stop=True,
                        )
                        if ch % 2 == 0:
                            nc.vector.tensor_copy(zB[:, sl], pb[:])
                        else:
                            nc.scalar.activation(zB[:, sl], pb[:], AF.Copy)

                    coltmp = colp.tile([128, 16], F32, name="coltmp")
                    mn_c = coltmp[:, 0:1]
                    mx_c = coltmp[:, 1:2]
                    nc.vector.tensor_reduce(
                        out=mn_c, in_=z32[:], axis=mybir.AxisListType.X, op=ALU.min
                    )
                    nc.vector.reduce_max(out=mx_c, in_=z32[:], axis=mybir.AxisListType.X)
                    lo0 = strip[0:1, 8:9]
                    hi0 = strip[0:1, 9:10]
                    col_reduce(mn_c, lo0, "min")
                    col_reduce(mx_c, hi0, "max")

                    N_ROUNDS = 3
                    lo_cur, hi_cur = lo0, hi0
                    si = 10
                    tau_col = coltmp[:, 2:3]
                    dB = coltmp[:, 3:4]
                    loB = coltmp[:, 4:5]
                    cnt_col = coltmp[:, 5:6]
                    sgn_col = coltmp[:, 6:7]
                    for r in range(N_ROUNDS):
                        d0 = strip[0:1, si : si + 1]
                        nc.vector.tensor_sub(d0, hi_cur, lo_cur)
                        pe_bcast_col(d0, dB)
                        pe_bcast_col(lo_cur, loB)
                        nc.vector.tensor_mul(tau_col, alphac[:], dB)
                        nc.vector.tensor_add(tau_col, tau_col, loB)
                        nc.vector.scalar_tensor_tensor(
                            out=scrB[:],
                            in0=zB[:],
                            scalar=tau_col,
                            in1=zB[:],
                            op0=ALU.is_gt,
                            op1=ALU.bypass,
                            accum_out=cnt_col,
                        )
                        nc.vector.tensor_scalar(
                            out=sgn_col, in0=cnt_col, scalar1=float(SLOTS),
                            scalar2=None, op0=ALU.is_ge,
                        )
                        pj = ps_small.tile([1, 1], F32, tag="pj")
                        nc.tensor.matmul(
                            pj[:], lhsT=sgn_col, rhs=onescol[:], start=True, stop=True
                        )
                        dd = strip[0:1, si + 1 : si + 2]
                        nc.vector.tensor_scalar(
                            out=dd, in0=d0, scalar1=1.0 / 128, scalar2=None,
                            op0=ALU.mult,
                        )
                        tmp = strip[0:1, si + 2 : si + 3]
                        nc.vector.tensor_mul(tmp, pj[:], dd)
                        lo_n = strip[0:1, si + 3 : si + 4]
                        nc.vector.tensor_add(lo_n, lo_cur, tmp)
                        hi_n = strip[0:1, si + 4 : si + 5]
                        nc.vector.tensor_add(hi_n, lo_n, dd)
                        lo_cur, hi_cur = lo_n, hi_n
                        si += 5

                    # v0 = min(z > lo_cur) exactly, from (128,32) cols
                    loB2 = coltmp[:, 7:8]
                    pe_bcast_col(lo_cur, loB2)
                    mask_u8 = colp.tile([128, 32], mybir.dt.uint8, name="mask_u8")
                    nc.vector.tensor_scalar(
                        out=mask_u8[:], in0=z32[:], scalar1=loB2, scalar2=None,
                        op0=ALU.is_gt,
                    )
                    w_c = colp.tile([128, 32], F32, name="w_c")
                    nc.vector.memset(w_c[:], BIG)
                    nc.vector.copy_predicated(w_c[:], mask_u8[:], z32[:])
                    wmin_c = coltmp[:, 8:9]
                    nc.vector.tensor_reduce(
                        out=wmin_c, in_=w_c[:], axis=mybir.AxisListType.X, op=ALU.min
                    )
                    v0 = strip[0:1, si : si + 1]
                    col_reduce(wmin_c, v0, "min")
                    # c2cnt = count(z > v0); need = 256 - c2cnt
                    vB = coltmp[:, 9:10]
                    pe_bcast_col(v0, vB)
                    gt32 = c32("gt32")
                    nc.vector.scalar_tensor_tensor(
                        out=gt32[:], in0=z32[:], scalar=vB, in1=z32[:],
                        op0=ALU.is_gt, op1=ALU.bypass, accum_out=coltmp[:, 10:11],
                    )
                    pc2 = ps_small.tile([1, 1], F32, tag="pj")
                    nc.tensor.matmul(
                        pc2[:], lhsT=coltmp[:, 10:11], rhs=onescol[:],
                        start=True, stop=True,
                    )
                    need0 = strip[0:1, si + 1 : si + 2]
                    nc.vector.tensor_scalar(
                        out=need0, in0=pc2[:], scalar1=float(SLOTS), scalar2=-1.0,
                        op0=ALU.subtract, op1=ALU.mult,
                    )
                    needcol = coltmp[:, 11:12]
                    pe_bcast_col(need0, needcol)

                    # tie-fix: prefix count of (z == v0) in T order
                    maskeq = c32("maskeq")
                    nc.vector.tensor_scalar(
                        out=maskeq[:], in0=z32[:], scalar1=vB, scalar2=None,
                        op0=ALU.is_equal,
                    )
                    iscan = c32("iscan")
                    nc.vector.tensor_tensor_scan(
                        iscan[:], maskeq[:], maskeq[:], 0.0,
                        op0=ALU.add, op1=ALU.bypass,
                    )
                    # exclusive partition offsets: scan of per-partition totals
                    ptot_tr = ps_small.tile([1, 128], F32, tag="tr")
                    nc.tensor.transpose(ptot_tr[:], iscan[:, 31:32], ident[:])
                    ptr_row = rowp.tile([1, 300], F32, name="ptr_row", tag="ptr_row")
                    nc.vector.tensor_copy(ptr_row[0:1, 0:128], ptot_tr[:])
                    nc.vector.tensor_tensor_scan(
                        ptr_row[0:1, 128:256], ptr_row[0:1, 0:128],
                        ptr_row[0:1, 0:128], 0.0, op0=ALU.add, op1=ALU.bypass,
                    )
                    nc.vector.tensor_sub(
                        ptr_row[0:1, 128:256], ptr_row[0:1, 128:256],
                        ptr_row[0:1, 0:128],
                    )
                    offs_ps = ps_small.tile([128, 1], F32, tag="bc")
                    nc.tensor.matmul(
                        offs_ps[:], lhsT=ptr_row[0:1, 128:256],
                        rhs=ones1x128[0:1, 0:1],
                        start=True, stop=True,
                    )
                    offcol = coltmp[:, 12:13]
                    nc.vector.tensor_copy(offcol, offs_ps[:])
                    incl = c32("incl")
                    nc.vector.tensor_scalar(
                        out=incl[:], in0=iscan[:], scalar1=offcol, scalar2=None,
                        op0=ALU.add,
                    )
                    fill = c32("fill")
                    nc.vector.scalar_tensor_tensor(
                        out=fill[:], in0=incl[:], scalar=needcol, in1=maskeq[:],
                        op0=ALU.is_le, op1=ALU.mult,
                    )
                    sel32 = c32("sel32")
                    nc.vector.tensor_scalar(
                        out=sel32[:], in0=z32[:], scalar1=vB, scalar2=None,
                        op0=ALU.is_gt,
                    )
                    nc.vector.tensor_add(sel32[:], sel32[:], fill[:])
                    nc.sync.dma_start(
                        dbg[1:2, :].rearrange("o (p c) -> (o p) c", p=128), sel32[:]
                    )

                    # -- masked softmax in (128,32) -----------------------
                    src = c32("src")
                    nc.scalar.dma_start(
                        src[:], sd[:].rearrange("o (p c) -> (o p) c", p=128)
                    )
                    s32 = c32("s32")
                    c56 = colp.tile([128, 2], F32, name="c56")
                    pe_bcast_col(strip[0:1, 5:7], c56[:])
                    ln_row_fix(s32, src, c56[:, 0:1], c56[:, 1:2])
                    nc.sync.dma_start(
                        dbg[2:3, :].rearrange("o (p c) -> (o p) c", p=128), s32[:]
                    )
                    masked = c32("masked")
                    nc.vector.tensor_scalar(
                        out=masked[:], in0=sel32[:], scalar1=-1.0, scalar2=BIG,
                        op0=ALU.add, op1=ALU.mult,
                    )
                    nc.vector.tensor_add(masked[:], masked[:], s32[:])
                    nc.vector.reduce_max(
                        out=coltmp[:, 13:14], in_=masked[:], axis=mybir.AxisListType.X
                    )
                    smax = strip[0:1, si + 2 : si + 3]
                    col_reduce(coltmp[:, 13:14], smax, "max")
                    nsmax = strip[0:1, si + 3 : si + 4]
                    nc.vector.tensor_scalar(
                        out=nsmax, in0=smax, scalar1=-1.0, scalar2=None, op0=ALU.mult
                    )
                    nsmaxcol = coltmp[:, 14:15]
                    pe_bcast_col(nsmax, nsmaxcol)
                    e32h = colp.tile([128, 32], F16, name="e32h")
                    zsumcol = colp.tile([128, 1], F32, name="zsumcol")
                    nc.scalar.activation(
                        e32h[:], masked[:], AF.Exp, bias=nsmaxcol,
                        accum_out=zsumcol[:, 0:1],
                    )
                    zsum = strip[0:1, si + 4 : si + 5]
                    col_reduce(zsumcol[:, 0:1], zsum, "sum")
                    rz = strip[0:1, si + 5 : si + 6]
                    nc.vector.reciprocal(out=rz, in_=zsum)
                    # ctx weights w = e * r ; sigma = sum(w * m)
                    w32h = colp.tile([128, 32], F16, name="w32h")
                    nc.vector.tensor_mul(w32h[:], e32h[:], r32[:])
                    sig32 = c32("sig32")
                    nc.vector.scalar_tensor_tensor(
                        out=sig32[:], in0=m32[:], scalar=1.0, in1=w32h[:],
                        op0=ALU.mult, op1=ALU.mult, accum_out=coltmp[:, 15:16],
                    )
                    sig = strip[0:1, si + 6 : si + 7]
                    col_reduce(coltmp[:, 15:16], sig, "sum")
                    nc.sync.dma_start(
                        wd[0:1, 0:T].rearrange("o (p c) -> (o p) c", p=128), w32h[:]
                    )

                    # broadcast w row and accumulate ctxraw over yT
                    wB = bigp.tile([128, T], F16, name="wB")
                    for ch in range(NCH):
                        sl = slice(ch * CW, (ch + 1) * CW)
                        wrc = rowp.tile([1, CW], F16, name="wrc", tag="wrc")
                        nc.sync.dma_start(wrc[:], wd[0:1, sl])
                        pb = ps_wide.tile([128, CW], F32, tag="pb")
                        nc.tensor.matmul(
                            pb[:], lhsT=ones1x128h[:], rhs=wrc[:],
                            start=True, stop=True,
                        )
                        if ch % 2 == 0:
                            nc.vector.tensor_copy(wB[:, sl], pb[:])
                        else:
                            nc.scalar.activation(wB[:, sl], pb[:], AF.Copy)
                    scrH = bigp.tile([128, T], F16, name="scrH")
                    for k in range(NK):
                        nc.vector.scalar_tensor_tensor(
                            out=scrH[:],
                            in0=yT[k][:],
                            scalar=1.0,
                            in1=wB[:],
                            op0=ALU.mult,
                            op1=ALU.mult,
                            accum_out=ctx4[:, k : k + 1],
                        )
                    # ctx = (g*ctxraw - sig*g) * rz + b
                    sigcol = coltmp[:, 13:14]
                    pe_bcast_col(sig, sigcol)
                    rzcol = coltmp[:, 14:15]
                    pe_bcast_col(rz, rzcol)
                    nc.vector.tensor_scalar(
                        out=ctx4[:], in0=ctx4[:], scalar1=sigcol, scalar2=None,
                        op0=ALU.subtract,
                    )
                    nc.vector.tensor_mul(ctx4[:], ctx4[:], gcol[:])
                    nc.vector.tensor_scalar(
                        out=ctx4[:], in0=ctx4[:], scalar1=rzcol, scalar2=None,
                        op0=ALU.mult,
                    )
                    nc.vector.tensor_add(ctx4[:], ctx4[:], bcol[:])
                # yT/phase-B pools closed

            # ---------------- allgather + output projection ----------------
            nc.sync.dma_start(cc_in[:], ctx4[:])
            nc.gpsimd.collective_compute(
                "AllGather",
                ALU.bypass,
                replica_groups=[list(range(NCORES))],
                ins=[cc_in[:]],
                outs=[cc_out[:]],
            )
            nc.sync.dma_start(
                ctxall[:].rearrange("p (j b) -> p j b", j=NK),
                cc_out[:].rearrange("(b p) j -> p j b", p=128),
            )
            ctxall_h = cpool.tile([128, 32], F16, name="ctxall_h")
            nc.vector.tensor_copy(ctxall_h[:], ctxall[:])
            nchunks = (VS + CW - 1) // CW
            with (
                tc.tile_pool(name="lo", bufs=4) as lopool,
                tc.tile_pool(name="psl", bufs=4, space="PSUM") as psl,
            ):
                for n in range(nchunks):
                    w = min(CW, VS - n * CW)
                    vsl = slice(n * CW, n * CW + w)
                    bt = lopool.tile([1, CW], F16, name="bo", tag="bo")
                    nc.sync.dma_start(bt[:, :w], bout[:, vsl])
                    pl = psl.tile([B, CW], F32, tag="pl")
                    for k in range(NK):
                        nc.tensor.matmul(
                            pl[:, :w],
                            lhsT=ctxall_h[:, 8 * k : 8 * (k + 1)],
                            rhs=wot[k][:, vsl],
                            start=(k == 0),
                            stop=False,
                        )
                    nc.tensor.matmul(
                        pl[:, :w], lhsT=ones1x8h[:], rhs=bt[:, :w],
                        start=False, stop=True,
                    )
                    lt = lopool.tile([B, CW], F32, name="lt", tag="lt")
                    nc.vector.tensor_copy(lt[:, :w], pl[:, :w])
                    nc.sync.dma_start(logits[:, vsl], lt[:, :w])

    return nc


def _host_prep(inputs):
    f32 = lambda a: np.ascontiguousarray(np.asarray(a, dtype=np.float32))
    f16c = lambda a: np.ascontiguousarray(np.asarray(a, dtype=np.float16))
    seq = np.asarray(inputs["seq"])
    embed = f32(inputs["embed"])
    w1 = f32(inputs["W1"])
    b1 = f32(inputs["b1"])
    w2 = f32(inputs["W2"])
    b2 = f32(inputs["b2"])
    ln_g = f32(inputs["ln_g"])
    ln_b = f32(inputs["ln_b"])
    gw = f32(inputs["gate_W"])
    gb = f32(inputs["gate_b"])
    qw = f32(inputs["q_W"])
    qb = f32(inputs["q_b"])
    wout_f = f32(inputs["out_W"])
    bout_f = f32(inputs["out_b"])

    colpack = lambda v: np.ascontiguousarray(
        v.reshape(-1, 128).T.astype(np.float32)
    )  # (Ntiles*128,) -> (128, Ntiles); tile k col = dims [128k, 128k+128)
    cnt = np.minimum(K, T - 1 - np.arange(T)).astype(np.float32)
    cntrec = np.zeros(T, dtype=np.float32)
    cntrec[cnt > 0] = 1.0 / cnt[cnt > 0]

    gw1 = gw[:D, 0]
    gw2 = gw[D:, 0]
    gw1g = (gw1 * ln_g).astype(np.float32)
    gw2g = (gw2 * ln_g).astype(np.float32)
    gw12 = np.concatenate(
        [colpack(gw1g), colpack(gw2g)], axis=1
    ).astype(np.float16)
    c1 = float(np.dot(gw1, ln_g))
    c2 = float(np.dot(gw1, ln_b))
    c3 = float(np.dot(gw2, ln_g))
    c4 = float(np.dot(gw2, ln_b))

    base = {
        "embed_h": f16c(embed),
        "w1h": f16c(w1),
        "w2h": f16c(w2),
        "qwh": f16c(qw),
        "b1c": colpack(b1),
        "b2c": colpack(b2),
        "gcol": colpack(ln_g),
        "bcol": colpack(ln_b),
        "gw12": np.ascontiguousarray(gw12),
        "qbc": colpack(qb),
        "ident": np.eye(128, dtype=np.float32),
        "onesc": np.ones((128, 1), dtype=np.float32),
        "ones1x128": np.ones((1, 128), dtype=np.float32),
        "ones1x128h": np.ones((1, 128), dtype=np.float16),
        "ones1x8h": np.ones((1, 8), dtype=np.float16),
        "alphac": ((np.arange(128, dtype=np.float32) + 1.0) / 128.0).reshape(128, 1),
        "cnt32": np.ascontiguousarray(cntrec.reshape(128, 32)),
        "sc_in": np.array(
            [[EPS, float(gb[0]), c1, c2, c3, c4, 0.0, 0.0]], dtype=np.float32
        ),
    }
    wout_pad = np.zeros((D, NCORES * VS), dtype=np.float32)
    wout_pad[:, :V] = wout_f
    bout_pad = np.zeros(NCORES * VS, dtype=np.float32)
    bout_pad[:V] = bout_f

    in_maps = []
    for c in range(NCORES):
        m = dict(base)
        m["seq_idx"] = np.ascontiguousarray(
            seq[c].reshape(32, 128).T.astype(np.int32)
        )
        m["wout"] = np.ascontiguousarray(
            wout_pad[:, c * VS : (c + 1) * VS].astype(np.float16)
        )
        m["bout"] = np.ascontiguousarray(
            bout_pad[c * VS : (c + 1) * VS].reshape(1, VS).astype(np.float16)
        )
        in_maps.append(m)
    return in_maps


def get_nc():
    key = (DEBUG_HT,)
    if key not in _cache:
        _cache[key] = build_bass()
    return _cache[key]


def run_full(inputs, trace=False):
    """Run the kernel; returns (logits_full, BassKernelResults)."""
    nc = get_nc()
    in_maps = _host_prep(inputs)
    res = run_bass_kernel_spmd(
        nc, in_maps, core_ids=list(range(NCORES)), trace=trace
    )
    parts = [res.results[c]["logits"] for c in range(NCORES)]
    logits = np.concatenate(parts, axis=1)[:, :V]
    return logits, res


def kernel(**inputs) -> np.ndarray:
    logits, _ = run_full(inputs, trace=False)
    return logits


# revision 33
# speedup vs baseline: 1.6893x; 1.0713x over previous
"""Trainium2 Bass kernel for nn_LookaheadModel (topk_masking).

Sharding: data-parallel over batch B=8 (one batch element per core) for the
encoder; tiny AllGather of per-batch context vectors; vocab-sharded output
projection (each core computes logits[:, shard]).

v2 design:
- fp16 matmul path (embed/W1/W2/qW/out_W in fp16) -> FWL weight loads hide
  LDWEIGHTS behind matmul streaming.
- XBAR DMA transpose for the token->feature-major flip (no PE transposes).
- LayerNorm folding: phase A stores yT = (h0+ff)^T (pre-LN, fp16) plus
  per-token row stats S1=sum(y), S2=sum(y^2) and gate rows aR=(g.gw1)^T y,
  bR=(g.gw2)^T y. All LN effects applied later as per-token scalar algebra:
  h = (y - m) * r * g + b_ln  =>  w^T h = r*(gw)^T y - r*m*(w^T g) + w^T b_ln.
- Phase B runs in (128,32) column layout (partition p holds tokens
  [32p, 32p+32)), with tiny DRAM roundtrips to re-layout rows.
- Exact top-256 via 3-round 128-way threshold search + tie fix (as v1).
- out_W prefetched into SBUF at kernel start; AllGather of ctx as in v1.

Self-contained: only needs numpy + the system-installed concourse package.
"""

import numpy as np

import bass_rust
import concourse.bass as bass
import concourse.mybir as mybir
from concourse.bass_utils import run_bass_kernel_spmd
from concourse.tile import TileContext

AF = mybir.ActivationFunctionType
ALU = mybir.AluOpType
F32 = mybir.dt.float32
F32R = mybir.dt.float32r
F16 = mybir.dt.float16
I32 = mybir.dt.int32

# ---------------------------------------------------------------------------
# Workaround: this walrus build rejects any instruction carrying more than one
# sync-wait command. Hoist excess waits onto same-engine NOPs (sequential on
# the same engine queue, so semantically identical).
# ---------------------------------------------------------------------------
_MAX_WAITS = 1
_nop_counter = [0]


def _split_waits_in_ordered(nc, ordered):
    for bb_name, insts in ordered.items():
        out = []
        for inst in insts:
            si = inst.sync_info
            waits = list(si.on_wait) if si and si.on_wait else []
            if len(waits) > _MAX_WAITS:
                spill, keep = waits[:-_MAX_WAITS], waits[-_MAX_WAITS:]
                for i in range(0, len(spill), _MAX_WAITS):
                    _nop_counter[0] += 1
                    nop = bass_rust.InstNoOp(name=f"WSPILL-{_nop_counter[0]}")
                    nop.engine = inst.engine
                    nop.sync_info = mybir.SyncInfo(
                        on_wait=list(spill[i : i + _MAX_WAITS]), on_update=[]
                    )
                    nop.bass_nofuse = True
                    nc.register_instruction(nop, overwrite=True)
                    out.append(nop)
                si.on_wait = keep
            out.append(inst)
        if len(out) != len(insts):
            insts[:] = out


_orig_lower = TileContext._lower_ordered_insts
_orig_drain = TileContext._drain_and_barrier


def _lower_with_split(self, ordered):
    _split_waits_in_ordered(self.nc, ordered)
    return _orig_lower(self, ordered)


def _drain_and_barrier_split(self, tick_clock, wait_clock):
    nc = self.nc
    sc = bass_rust.ScopedClock({None: tick_clock.global_clock})
    drain_inst = nc.sync.drain()
    wait_clock.add_sem_waits(drain_inst.ins, sc)
    si = drain_inst.ins.sync_info
    waits = list(si.on_wait or [])
    if len(waits) > _MAX_WAITS:
        si.on_wait = waits[:_MAX_WAITS]
        rest = waits[_MAX_WAITS:]
        for i in range(0, len(rest), _MAX_WAITS):
            nop = nc.sync.nop(nofuse=True, hint=f"drain_wait_spill_{i}")
            nop.ins.sync_info = mybir.SyncInfo(
                on_wait=list(rest[i : i + _MAX_WAITS]), on_update=[]
            )
    nc.all_engine_barrier()
    popped = nc._tile_sem_poison_stack.pop()
    assert popped is self._sem_poison
    nc.clear_and_free_semaphores(list(self.sems.allocated().values()))
    nc.all_engine_barrier()


def _apply_patch():
    TileContext._drain_and_barrier = _drain_and_barrier_split
    TileContext._lower_ordered_insts = _lower_with_split


# ---------------------------------------------------------------------------
# Problem constants
# ---------------------------------------------------------------------------
V, D, SLOTS, K = 50257, 512, 256, 8
B, T = 8, 4096
NCORES = 8
VS = 6283  # vocab shard width per core; 8*6283 = 50264 >= V
NCH = 8  # T chunks of width 512
CW = 512
NK = D // 128  # 4 feature tiles
NF = 2 * D // 128  # 8 hidden tiles
BIG = 1.0e30
EPS = 1e-5
TPAD = T + 128  # padded row length for shifted window loads

PURE_FP32 = False  # kept for test.py compat; ignored (always fp16 path)
DEBUG_HT = False  # adds a (D, T) dump of yT per core (bring-up only)

_cache = {}


def build_bass():
    _apply_patch()
    nc = bass.Bass(trn_type="TRN2", num_devices=NCORES)

    # ---- I/O ----
    embed_h = nc.dram_tensor("embed_h", (V, D), F16, kind="ExternalInput")
    seq_idx = nc.dram_tensor("seq_idx", (128, 32), I32, kind="ExternalInput")
    w1h_d = nc.dram_tensor("w1h", (D, 2 * D), F16, kind="ExternalInput")
    w2h_d = nc.dram_tensor("w2h", (2 * D, D), F16, kind="ExternalInput")
    qwh_d = nc.dram_tensor("qwh", (D, D), F16, kind="ExternalInput")
    b1c = nc.dram_tensor("b1c", (128, NF), F32, kind="ExternalInput")
    b2c = nc.dram_tensor("b2c", (128, NK), F32, kind="ExternalInput")
    gcol_d = nc.dram_tensor("gcol", (128, NK), F32, kind="ExternalInput")
    bcol_d = nc.dram_tensor("bcol", (128, NK), F32, kind="ExternalInput")
    rows3_d = nc.dram_tensor("rows3", (128, 3 * NK), F16, kind="ExternalInput")
    qbc = nc.dram_tensor("qbc", (128, NK), F32, kind="ExternalInput")
    ident_in = nc.dram_tensor("ident", (128, 128), F32, kind="ExternalInput")
    onesc_in = nc.dram_tensor("onesc", (128, 1), F32, kind="ExternalInput")
    ones1x128_in = nc.dram_tensor("ones1x128", (1, 128), F32, kind="ExternalInput")
    ones1x8h_in = nc.dram_tensor("ones1x8h", (1, 8), F16, kind="ExternalInput")
    ones1x128h_in = nc.dram_tensor("ones1x128h", (1, 128), F16, kind="ExternalInput")
    alpha_in = nc.dram_tensor("alphac", (128, 1), F32, kind="ExternalInput")
    cnt32_in = nc.dram_tensor("cnt32", (128, 32), F32, kind="ExternalInput")
    # eps, gate_b, c1=gw1.g, c2=gw1.b, c3=gw2.g, c4=gw2.b, 0, 0
    sc_in = nc.dram_tensor("sc_in", (1, 8), F32, kind="ExternalInput")
    wout = nc.dram_tensor("wout", (D, VS), F16, kind="ExternalInput")
    bout = nc.dram_tensor("bout", (1, VS), F16, kind="ExternalInput")

    logits = nc.dram_tensor("logits", (B, VS), F32, kind="ExternalOutput")
    dbg = nc.dram_tensor("dbg", (5, T), F32, kind="ExternalOutput")
    if DEBUG_HT:
        ytdump = nc.dram_tensor("ytdump", (D, T), F32, kind="ExternalOutput")

    # DRAM scratch rows
    s1d = nc.dram_tensor("s1d", (1, T), F32, kind="Internal")
    s2d = nc.dram_tensor("s2d", (1, T), F32, kind="Internal")
    ard = nc.dram_tensor("ard", (1, T), F32, kind="Internal")
    brd = nc.dram_tensor("brd", (1, T), F32, kind="Internal")
    bfd = nc.dram_tensor("bfd", (1, TPAD), F32, kind="Internal")
    zd = nc.dram_tensor("zd", (1, T), F32, kind="Internal")
    sd = nc.dram_tensor("sd", (1, T), F32, kind="Internal")
    wd = nc.dram_tensor("wd", (1, T), F16, kind="Internal")

    cc_in = nc.dram_tensor("cc_in", (128, NK), F32, kind="Internal")
    cc_out = nc.dram_tensor(
        "cc_out", (128 * NCORES, NK), F32, kind="Internal", addr_space="Shared"
    )

    with TileContext(nc) as tc:
        with tc.tile_pool(name="consts", bufs=1) as cpool:
            # ---------------- persistent constants / weight prefetch -------
            # out_W on the scalar HWDGE queue: big transfer, overlaps phase A
            # without blocking the sync queue.
            wot = []
            for k in range(NK):
                wt = cpool.tile([128, VS], F16, name=f"wot{k}")
                nc.scalar.dma_start(wt[:], wout[128 * k : 128 * (k + 1), :])
                wot.append(wt)


            w1t = []
            for k in range(NK):
                wt = cpool.tile([128, 2 * D], F16, name=f"w1t{k}")
                nc.sync.dma_start(wt[:], w1h_d[128 * k : 128 * (k + 1), :])
                w1t.append(wt)
            w2t = []
            for k in range(NF):
                wt = cpool.tile([128, D], F16, name=f"w2t{k}")
                nc.sync.dma_start(wt[:], w2h_d[128 * k : 128 * (k + 1), :])
                w2t.append(wt)
            qwt = []
            for k in range(NK):
                wt = cpool.tile([128, D], F16, name=f"qwt{k}")
                nc.sync.dma_start(wt[:], qwh_d[128 * k : 128 * (k + 1), :])
                qwt.append(wt)

            ident = cpool.tile([128, 128], F32, name="ident_t")
            nc.sync.dma_start(ident[:], ident_in[:])
            ident_h = cpool.tile([128, 128], F16, name="ident_h")
            nc.vector.tensor_copy(ident_h[:], ident[:])
            b1t = cpool.tile([128, NF], F32, name="b1t")
            nc.sync.dma_start(b1t[:], b1c[:])
            b2t = cpool.tile([128, NK], F32, name="b2t")
            nc.sync.dma_start(b2t[:], b2c[:])
            gcol = cpool.tile([128, NK], F32, name="gcol_t")
            nc.sync.dma_start(gcol[:], gcol_d[:])
            bcol = cpool.tile([128, NK], F32, name="bcol_t")
            nc.sync.dma_start(bcol[:], bcol_d[:])
            rows3 = cpool.tile([128, 3 * NK], F16, name="rows3_t")
            nc.sync.dma_start(rows3[:], rows3_d[:])
            qbt = cpool.tile([128, NK], F32, name="qbt")
            nc.sync.dma_start(qbt[:], qbc[:])
            onescol = cpool.tile([128, 1], F32, name="onescol")
            nc.sync.dma_start(onescol[:], onesc_in[:])
            onescol_r = cpool.tile([128, 1], F32R, name="onescol_r")
            nc.vector.tensor_copy(onescol_r[:], onescol[:])
            onescol_h = cpool.tile([128, 1], F16, name="onescol_h")
            nc.vector.tensor_copy(onescol_h[:], onescol[:])
            ones1x128 = cpool.tile([1, 128], F32, name="ones1x128")
            nc.sync.dma_start(ones1x128[:], ones1x128_in[:])
            ones1x128h = cpool.tile([1, 128], F16, name="ones1x128h")
            nc.sync.dma_start(ones1x128h[:], ones1x128h_in[:])
            ones1x8h = cpool.tile([1, 8], F16, name="ones1x8h")
            nc.sync.dma_start(ones1x8h[:], ones1x8h_in[:])
            alphac = cpool.tile([128, 1], F32, name="alphac_t")
            nc.sync.dma_start(alphac[:], alpha_in[:])
            cnt32 = cpool.tile([128, 32], F32, name="cnt32_t")
            nc.sync.dma_start(cnt32[:], cnt32_in[:])
            scin = cpool.tile([1, 8], F32, name="scin")
            nc.sync.dma_start(scin[:], sc_in[:])
            eps_ap = scin[0:1, 0:1]
            gb_ap = scin[0:1, 1:2]
            c1_ap = scin[0:1, 2:3]
            c2_ap = scin[0:1, 3:4]
            c3_ap = scin[0:1, 4:5]
            c4_ap = scin[0:1, 5:6]
            sidx = cpool.tile([128, 32], I32, name="sidx")
            nc.sync.dma_start(sidx[:], seq_idx[:])
            zpad = cpool.tile([1, 128], F32, name="zpad")
            nc.vector.memset(zpad[:], 0.0)
            nc.sync.dma_start(bfd[0:1, T:TPAD], zpad[:])

            strip = cpool.tile([1, 64], F32, name="strip")
            ctx4 = cpool.tile([128, NK], F32, name="ctx4")
            ctxall = cpool.tile([128, 32], F32, name="ctxall")

            with tc.tile_pool(name="yT", bufs=1) as ypool_p:
                yT = [ypool_p.tile([128, T], F16, name=f"yT{k}") for k in range(NK)]

                # ---------------- phase A: gather + FFN (pre-LN) ----------
                with (
                    tc.tile_pool(name="gat", bufs=3) as gpool,
                    tc.tile_pool(name="x0p", bufs=2) as x0pool,
                    tc.tile_pool(name="ap", bufs=2) as apool,
                    tc.tile_pool(name="sqp", bufs=2) as sqpool,
                    tc.tile_pool(name="strp", bufs=2) as strpool,
                    tc.tile_pool(name="pstp", bufs=2, space="PSUM") as pstp,
                    tc.tile_pool(name="psa", bufs=2, space="PSUM") as psa,
                    tc.tile_pool(name="psf", bufs=2, space="PSUM") as psf,
                    tc.tile_pool(name="psr", bufs=1, space="PSUM") as psr,
                ):
                    # software-pipelined: rows(ch-1) interleaves with L1(ch)
                    def do_rows_a(ch):
                        # [S1; aR; bR] = [ones|gw1g|gw2g]^T y in one group
                        sl = slice(ch * CW, (ch + 1) * CW)
                        rp_sab = psr.tile([3, CW], F32, tag="rs1")
                        for m in range(NK):
                            nc.tensor.matmul(
                                rp_sab[:],
                                lhsT=rows3[:, 3 * m : 3 * (m + 1)],
                                rhs=yT[m][:, sl],
                                start=(m == 0),
                                stop=(m == NK - 1),
                            )
                        strp = strpool.tile(
                            [3, 2 * CW], F32, name="strp", tag="strp"
                        )
                        nc.vector.tensor_copy(strp[:, 0:CW], rp_sab[:])
                        nc.sync.dma_start(s1d[0:1, sl], strp[0:1, 0:CW])
                        nc.sync.dma_start(ard[0:1, sl], strp[1:2, 0:CW])
                        nc.sync.dma_start(brd[0:1, sl], strp[2:3, 0:CW])
                        return strp

                    def do_rows_b(ch, strp):
                        # S2 = ones^T y^2 (sq from ACT)
                        sl = slice(ch * CW, (ch + 1) * CW)
                        rp_s2 = psr.tile([1, CW], F32, tag="rs2")
                        for m in range(NK):
                            sq = sqpool.tile([128, CW], F16, name="sq", tag="sq")
                            nc.scalar.activation(sq[:], yT[m][:, sl], AF.Square)
                            nc.tensor.matmul(
                                rp_s2[:],
                                lhsT=onescol_h[:],
                                rhs=sq[:],
                                start=(m == 0),
                                stop=(m == NK - 1),
                            )
                        nc.vector.tensor_copy(strp[0:1, CW : 2 * CW], rp_s2[:])
                        nc.sync.dma_start(s2d[0:1, sl], strp[0:1, CW : 2 * CW])

                    prev = None  # (ch, strp) pending row-stats for prev chunk
                    for ch in range(NCH):
                        sl = slice(ch * CW, (ch + 1) * CW)
                        x0 = [
                            x0pool.tile([128, CW], F16, name=f"x0_{k}", tag=f"x0_{k}")
                            for k in range(NK)
                        ]
                        # gather + PE transpose (fp16, 1 cycle/row)
                        for blk in range(4):
                            tb = 4 * ch + blk
                            g = gpool.tile([128, D], F16, name="g", tag="g")
                            nc.gpsimd.indirect_dma_start(
                                out=g[:],
                                out_offset=None,
                                in_=embed_h[:],
                                in_offset=bass.IndirectOffsetOnAxis(
                                    ap=sidx[:, tb : tb + 1], axis=0
                                ),
                            )
                            tp = pstp.tile([128, D], F16, tag="tp")
                            for k in range(NK):
                                nc.tensor.transpose(
                                    tp[:, 128 * k : 128 * (k + 1)],
                                    g[:, 128 * k : 128 * (k + 1)],
                                    ident_h[:],
                                )
                            for k in range(NK):
                                dst = x0[k][:, 128 * blk : 128 * (blk + 1)]
                                src = tp[:, 128 * k : 128 * (k + 1)]
                                if k % 2 == 0:
                                    nc.vector.tensor_copy(dst, src)
                                else:
                                    nc.scalar.activation(dst, src, AF.Copy)
                        if prev is not None:
                            prev_strp = do_rows_a(prev)
                        # layer 1 + relu
                        af = [
                            apool.tile([128, CW], F16, name=f"af{m}", tag=f"af{m}")
                            for m in range(NF)
                        ]
                        for m in range(NF):
                            ps = psa.tile([128, CW], F32, tag="psa")
                            for k in range(NK):
                                nc.tensor.matmul(
                                    ps[:],
                                    lhsT=w1t[k][:, 128 * m : 128 * (m + 1)],
                                    rhs=x0[k][:],
                                    start=(k == 0),
                                    stop=(k == NK - 1),
                                )
                            nc.scalar.activation(
                                af[m][:], ps[:], AF.Relu, bias=b1t[:, m : m + 1]
                            )
                            if m == 3 and prev is not None:
                                do_rows_b(prev, prev_strp)
                                prev = None
                        # layer 2 + bias + residual -> yT (fp16)
                        for m in range(NK):
                            ps = psf.tile([128, CW], F32, tag="psf")
                            for k in range(NF):
                                nc.tensor.matmul(
                                    ps[:],
                                    lhsT=w2t[k][:, 128 * m : 128 * (m + 1)],
                                    rhs=af[k][:],
                                    start=(k == 0),
                                    stop=(k == NF - 1),
                                )
                            nc.vector.scalar_tensor_tensor(
                                out=yT[m][:, sl],
                                in0=ps[:],
                                scalar=b2t[:, m : m + 1],
                                in1=x0[m][:],
                                op0=ALU.add,
                                op1=ALU.add,
                            )
                        prev = ch
                    # drain last chunk's rows
                    last_strp = do_rows_a(NCH - 1)
                    do_rows_b(NCH - 1, last_strp)

                    if DEBUG_HT:
                        for k in range(NK):
                            ydf = sqpool.tile([128, T], F32, name=f"ydf{k}", tag="ydf")
                            nc.vector.tensor_copy(ydf[:], yT[k][:])
                            nc.sync.dma_start(
                                ytdump[128 * k : 128 * (k + 1), :], ydf[:]
                            )

                # ---------------- phase B ------------------------------
                with (
                    tc.tile_pool(name="colp", bufs=1) as colp,
                    tc.tile_pool(name="shp", bufs=1) as shp,
                    tc.tile_pool(name="rowp", bufs=2) as rowp,
                    tc.tile_pool(name="bigp", bufs=1) as bigp,
                    tc.tile_pool(name="pssm", bufs=1, space="PSUM") as ps_small,
                    tc.tile_pool(name="psrow", bufs=2, space="PSUM") as ps_row,
                    tc.tile_pool(name="pswd", bufs=2, space="PSUM") as ps_wide,
                ):
                    # -- small helpers ------------------------------------
                    def pe_bcast_col(src1x, dst_cols):
                        # broadcast a (1,n) row to (128,n) columns
                        n = src1x.shape[-1]
                        p = ps_small.tile([128, 8], F32, tag="bc")
                        nc.tensor.matmul(
                            p[:, :n], lhsT=ones1x128[:], rhs=src1x,
                            start=True, stop=True,
                        )
                        nc.vector.tensor_copy(dst_cols, p[:, :n])

                    # static scalars -> columns in one broadcast:
                    # 0=eps 1=gate_b 2=c1 3=c2 4=c3 5=c4
                    ccols8 = colp.tile([128, 8], F32, name="ccols8")
                    pe_bcast_col(scin[0:1, 0:8], ccols8[:])

                    def col_reduce(src_col, dst11, op):
                        # reduce a (128,1) column to (1,1): transpose + reduce
                        p = ps_small.tile([1, 128], F32, tag="tr")
                        nc.tensor.transpose(p[:], src_col, ident[:])
                        if op == "max":
                            nc.vector.reduce_max(
                                out=dst11, in_=p[:], axis=mybir.AxisListType.X
                            )
                        elif op == "min":
                            nc.vector.tensor_reduce(
                                out=dst11, in_=p[:], axis=mybir.AxisListType.X,
                                op=ALU.min,
                            )
                        else:
                            nc.vector.tensor_reduce(
                                out=dst11, in_=p[:], axis=mybir.AxisListType.X,
                                op=ALU.add,
                            )

                    # -- q vector (exact h at t=T-1) ----------------------
                    # m,r for last token from S1/S2 scalars
                    s1l = colp.tile([1, 2], F32, name="s1l")
                    nc.sync.dma_start(s1l[0:1, 0:1], s1d[0:1, T - 1 : T])
                    nc.sync.dma_start(s1l[0:1, 1:2], s2d[0:1, T - 1 : T])
                    ml_ap = strip[0:1, 0:1]
                    nc.vector.tensor_scalar(
                        out=ml_ap, in0=s1l[0:1, 0:1], scalar1=1.0 / D, scalar2=None,
                        op0=ALU.mult,
                    )
                    e2l = strip[0:1, 2:3]
                    nc.vector.tensor_scalar(
                        out=e2l, in0=s1l[0:1, 1:2], scalar1=1.0 / D, scalar2=None,
                        op0=ALU.mult,
                    )
                    varl = strip[0:1, 3:4]
                    tmpl = strip[0:1, 4:5]
                    nc.vector.tensor_mul(tmpl, ml_ap, ml_ap)
                    nc.vector.tensor_sub(varl, e2l, tmpl)
                    nc.scalar.activation(tmpl, varl, AF.Ln, bias=eps_ap)
                    rl_ap = strip[0:1, 1:2]
                    nc.scalar.activation(rl_ap, tmpl, AF.Exp, scale=-0.5)

                    mrlc = colp.tile([128, 2], F32, name="mrlc")
                    pe_bcast_col(strip[0:1, 0:2], mrlc[:])
                    mlc = mrlc[:, 0:1]
                    rlc = mrlc[:, 1:2]

                    ylast = colp.tile([128, NK], F32, name="ylast")
                    for k in range(NK):
                        nc.vector.tensor_copy(
                            ylast[:, k : k + 1], yT[k][:, T - 1 : T]
                        )
                    hlast = colp.tile([128, NK], F32, name="hlast")
                    # h = (y - m) * r * g + b
                    nc.vector.tensor_scalar(
                        out=hlast[:], in0=ylast[:], scalar1=mlc,
                        scalar2=None, op0=ALU.subtract,
                    )
                    nc.vector.tensor_scalar(
                        out=hlast[:], in0=hlast[:], scalar1=rlc,
                        scalar2=None, op0=ALU.mult,
                    )
                    nc.vector.tensor_mul(hlast[:], hlast[:], gcol[:])
                    nc.vector.tensor_add(hlast[:], hlast[:], bcol[:])
                    hlast_h = colp.tile([128, NK], F16, name="hlast_h")
                    nc.vector.tensor_copy(hlast_h[:], hlast[:])

                    qh = colp.tile([128, NK], F32, name="qh")
                    for j in range(NK):
                        pq = ps_small.tile([128, 1], F32, tag="pq")
                        for k in range(NK):
                            nc.tensor.matmul(
                                pq[:],
                                lhsT=qwt[k][:, 128 * j : 128 * (j + 1)],
                                rhs=hlast_h[:, k : k + 1],
                                start=(k == 0),
                                stop=(k == NK - 1),
                            )
                        nc.vector.tensor_add(qh[:, j : j + 1], pq[:], qbt[:, j : j + 1])
                    # qg = q * g (for score matmuls); c5 = q.g ; c6 = q.b
                    qg = colp.tile([128, NK], F32, name="qg")
                    nc.vector.tensor_mul(qg[:], qh[:], gcol[:])
                    qgh = colp.tile([128, NK], F16, name="qgh")
                    nc.vector.tensor_copy(qgh[:], qg[:])
                    dotc = colp.tile([128, 1], F32, name="dotc")
                    scr4 = colp.tile([128, NK], F32, name="scr4")
                    nc.vector.scalar_tensor_tensor(
                        out=scr4[:], in0=qh[:], scalar=1.0, in1=gcol[:],
                        op0=ALU.mult, op1=ALU.mult, accum_out=dotc[:, 0:1],
                    )
                    c5_ap = strip[0:1, 5:6]
                    col_reduce(dotc[:, 0:1], c5_ap, "sum")
                    nc.vector.scalar_tensor_tensor(
                        out=scr4[:], in0=qh[:], scalar=1.0, in1=bcol[:],
                        op0=ALU.mult, op1=ALU.mult, accum_out=dotc[:, 0:1],
                    )
                    c6_ap = strip[0:1, 6:7]
                    col_reduce(dotc[:, 0:1], c6_ap, "sum")

                    # -- raw score row: sR = (qg)^T y, per chunk ----------
                    for ch in range(NCH):
                        sl = slice(ch * CW, (ch + 1) * CW)
                        pss = ps_row.tile([1, CW], F32, tag="pss")
                        for k in range(NK):
                            nc.tensor.matmul(
                                pss[:],
                                lhsT=qgh[:, k : k + 1],
                                rhs=yT[k][:, sl],
                                start=(k == 0),
                                stop=(k == NK - 1),
                            )
                        sev = rowp.tile([1, CW], F32, name="sev", tag="sev")
                        nc.scalar.activation(sev[:], pss[:], AF.Copy)
                        nc.sync.dma_start(sd[0:1, sl], sev[:])

                    # -- per-token stats in (128,32) layout ---------------
                    c32 = lambda nm: colp.tile([128, 32], F32, name=nm)
                    s1c = c32("s1c")
                    nc.scalar.dma_start(
                        s1c[:], s1d[:].rearrange("o (p c) -> (o p) c", p=128)
                    )
                    s2c = c32("s2c")
                    nc.scalar.dma_start(
                        s2c[:], s2d[:].rearrange("o (p c) -> (o p) c", p=128)
                    )
                    arc = c32("arc")
                    nc.scalar.dma_start(
                        arc[:], ard[:].rearrange("o (p c) -> (o p) c", p=128)
                    )
                    brc = c32("brc")
                    nc.scalar.dma_start(
                        brc[:], brd[:].rearrange("o (p c) -> (o p) c", p=128)
                    )
                    m32 = c32("m32")
                    nc.vector.tensor_scalar(
                        out=m32[:], in0=s1c[:], scalar1=1.0 / D, scalar2=None,
                        op0=ALU.mult,
                    )
                    var32 = c32("var32")
                    nc.vector.tensor_scalar(
                        out=var32[:], in0=s2c[:], scalar1=1.0 / D, scalar2=None,
                        op0=ALU.mult,
                    )
                    t32 = c32("t32")
                    nc.vector.tensor_mul(t32[:], m32[:], m32[:])
                    nc.vector.tensor_sub(var32[:], var32[:], t32[:])
                    nc.vector.tensor_scalar(
                        out=t32[:], in0=var32[:], scalar1=ccols8[:, 0:1],
                        scalar2=None, op0=ALU.add,
                    )
                    ln32 = c32("ln32")
                    nc.scalar.activation(ln32[:], t32[:], AF.Ln)
                    r32 = c32("r32")
                    nc.scalar.activation(r32[:], ln32[:], AF.Exp, scale=-0.5)
                    rm32 = c32("rm32")
                    nc.vector.tensor_mul(rm32[:], r32[:], m32[:])

                    # gate rows -> a32, b32:  x = r*xR - rm*cG + cB

                    def ln_row_fix(out32, raw32, cg, cb):
                        nc.vector.tensor_mul(out32[:], raw32[:], r32[:])
                        nc.vector.tensor_scalar(
                            out=t32[:], in0=rm32[:], scalar1=cg, scalar2=None,
                            op0=ALU.mult,
                        )
                        nc.vector.tensor_sub(out32[:], out32[:], t32[:])
                        nc.vector.tensor_scalar(
                            out=out32[:], in0=out32[:], scalar1=cb, scalar2=None,
                            op0=ALU.add,
                        )

                    a32 = c32("a32")
                    ln_row_fix(a32, arc, ccols8[:, 2:3], ccols8[:, 3:4])
                    b32 = c32("b32")
                    ln_row_fix(b32, brc, ccols8[:, 4:5], ccols8[:, 5:6])

                    # windowed lookahead sum: u_t = sum_{s=1..8} b_{t+s}
                    nc.sync.dma_start(
                        bfd[0:1, 0:T].rearrange("o (p c) -> (o p) c", p=128), b32[:]
                    )
                    bsh = []
                    for s in range(1, K + 1):
                        bt_ = shp.tile([128, 32], F32, name=f"bsh{s}", tag=f"bsh{s}")
                        nc.scalar.dma_start(
                            bt_[:],
                            bfd[0:1, s : s + T].rearrange(
                                "o (p c) -> (o p) c", p=128
                            ),
                        )
                        bsh.append(bt_)
                    u32 = c32("u32")
                    nc.vector.tensor_add(u32[:], bsh[0][:], bsh[1][:])
                    for s in range(2, K):
                        nc.vector.tensor_add(u32[:], u32[:], bsh[s][:])
                    # z = a + gate_b + u * cntrec
                    z32 = c32("z32")
                    nc.vector.tensor_mul(t32[:], u32[:], cnt32[:])
                    nc.vector.tensor_add(z32[:], a32[:], t32[:])
                    nc.vector.tensor_scalar(
                        out=z32[:], in0=z32[:], scalar1=ccols8[:, 1:2],
                        scalar2=None, op0=ALU.add,
                    )
                    nc.sync.dma_start(
                        dbg[0:1, :].rearrange("o (p c) -> (o p) c", p=128), z32[:]
                    )
                    nc.sync.dma_start(
                        zd[0:1, 0:T].rearrange("o (p c) -> (o p) c", p=128), z32[:]
                    )

                    # -- selection: 3-round 128-way threshold search ------
                    zB = bigp.tile([128, T], F32, name="zB")
                    scrB = bigp.tile([128, T], mybir.dt.uint8, name="scrB")
                    for ch in range(NCH):
                        sl = slice(ch * CW, (ch + 1) * CW)
                        zrc = rowp.tile([1, CW], F32, name="zrc", tag="zrc")
                        nc.sync.dma_start(zrc[:], zd[0:1, sl])
                        pb = ps_wide.tile([128, CW], F32, tag="pb")
                        nc.tensor.matmul(
                            pb[:], lhsT=ones1x128[:], rhs=zrc[:],
                            start=True, stop=True,
                        )
                        if ch % 2 == 0:
                            nc.vector.tensor_copy(zB[:, sl], pb[:])
                        else:
                            nc.scalar.activation(zB[:, sl], pb[:], AF.Copy)

                    coltmp = colp.tile([128, 16], F32, name="coltmp")
                    mn_c = coltmp[:, 0:1]
                    mx_c = coltmp[:, 1:2]
                    nc.vector.tensor_reduce(
                        out=mn_c, in_=z32[:], axis=mybir.AxisListType.X, op=ALU.min
                    )
                    nc.vector.reduce_max(out=mx_c, in_=z32[:], axis=mybir.AxisListType.X)
                    lo0 = strip[0:1, 8:9]
                    hi0 = strip[0:1, 9:10]
                    col_reduce(mn_c, lo0, "min")
                    col_reduce(mx_c, hi0, "max")

                    N_ROUNDS = 3
                    lo_cur, hi_cur = lo0, hi0
                    si = 10
                    tau_col = coltmp[:, 2:3]
                    dB = coltmp[:, 3:4]
                    loB = coltmp[:, 4:5]
                    cnt_col = coltmp[:, 5:6]
                    sgn_col = coltmp[:, 6:7]
                    for r in range(N_ROUNDS):
                        d0 = strip[0:1, si : si + 1]
                        nc.vector.tensor_sub(d0, hi_cur, lo_cur)
                        pe_bcast_col(d0, dB)
                        pe_bcast_col(lo_cur, loB)
                        nc.vector.tensor_mul(tau_col, alphac[:], dB)
                        nc.vector.tensor_add(tau_col, tau_col, loB)
                        nc.vector.scalar_tensor_tensor(
                            out=scrB[:],
                            in0=zB[:],
                            scalar=tau_col,
                            in1=zB[:],
                            op0=ALU.is_gt,
                            op1=ALU.bypass,
                            accum_out=cnt_col,
                        )
                        nc.vector.tensor_scalar(
                            out=sgn_col, in0=cnt_col, scalar1=float(SLOTS),
                            scalar2=None, op0=ALU.is_ge,
                        )
                        pj = ps_small.tile([1, 1], F32, tag="pj")
                        nc.tensor.matmul(
                            pj[:], lhsT=sgn_col, rhs=onescol[:], start=True, stop=True
                        )
                        dd = strip[0:1, si + 1 : si + 2]
                        nc.vector.tensor_scalar(
                            out=dd, in0=d0, scalar1=1.0 / 128, scalar2=None,
                            op0=ALU.mult,
                        )
                        tmp = strip[0:1, si + 2 : si + 3]
                        nc.vector.tensor_mul(tmp, pj[:], dd)
                        lo_n = strip[0:1, si + 3 : si + 4]
                        nc.vector.tensor_add(lo_n, lo_cur, tmp)
                        hi_n = strip[0:1, si + 4 : si + 5]
                        nc.vector.tensor_add(hi_n, lo_n, dd)
                        lo_cur, hi_cur = lo_n, hi_n
                        si += 5

                    # v0 = min(z > lo_cur) exactly, from (128,32) cols
                    loB2 = coltmp[:, 7:8]
                    pe_bcast_col(lo_cur, loB2)
                    mask_u8 = colp.tile([128, 32], mybir.dt.uint8, name="mask_u8")
                    nc.vector.tensor_scalar(
                        out=mask_u8[:], in0=z32[:], scalar1=loB2, scalar2=None,
                        op0=ALU.is_gt,
                    )
                    w_c = colp.tile([128, 32], F32, name="w_c")
                    nc.vector.memset(w_c[:], BIG)
                    nc.vector.copy_predicated(w_c[:], mask_u8[:], z32[:])
                    wmin_c = coltmp[:, 8:9]
                    nc.vector.tensor_reduce(
                        out=wmin_c, in_=w_c[:], axis=mybir.AxisListType.X, op=ALU.min
                    )
                    v0 = strip[0:1, si : si + 1]
                    col_reduce(wmin_c, v0, "min")
                    # c2cnt = count(z > v0); need = 256 - c2cnt
                    vB = coltmp[:, 9:10]
                    pe_bcast_col(v0, vB)
                    gt32 = c32("gt32")
                    nc.vector.scalar_tensor_tensor(
                        out=gt32[:], in0=z32[:], scalar=vB, in1=z32[:],
                        op0=ALU.is_gt, op1=ALU.bypass, accum_out=coltmp[:, 10:11],
                    )
                    pc2 = ps_small.tile([1, 1], F32, tag="pj")
                    nc.tensor.matmul(
                        pc2[:], lhsT=coltmp[:, 10:11], rhs=onescol[:],
                        start=True, stop=True,
                    )
                    need0 = strip[0:1, si + 1 : si + 2]
                    nc.vector.tensor_scalar(
                        out=need0, in0=pc2[:], scalar1=float(SLOTS), scalar2=-1.0,
                        op0=ALU.subtract, op1=ALU.mult,
                    )
                    needcol = coltmp[:, 11:12]
                    pe_bcast_col(need0, needcol)

                    # tie-fix: prefix count of (z == v0) in T order
                    maskeq = c32("maskeq")
                    nc.vector.tensor_scalar(
                        out=maskeq[:], in0=z32[:], scalar1=vB, scalar2=None,
                        op0=ALU.is_equal,
                    )
                    iscan = c32("iscan")
                    nc.vector.tensor_tensor_scan(
                        iscan[:], maskeq[:], maskeq[:], 0.0,
                        op0=ALU.add, op1=ALU.bypass,
                    )
                    # exclusive partition offsets: scan of per-partition totals
                    ptot_tr = ps_small.tile([1, 128], F32, tag="tr")
                    nc.tensor.transpose(ptot_tr[:], iscan[:, 31:32], ident[:])
                    ptr_row = rowp.tile([1, 300], F32, name="ptr_row", tag="ptr_row")
                    nc.vector.tensor_copy(ptr_row[0:1, 0:128], ptot_tr[:])
                    nc.vector.tensor_tensor_scan(
                        ptr_row[0:1, 128:256], ptr_row[0:1, 0:128],
                        ptr_row[0:1, 0:128], 0.0, op0=ALU.add, op1=ALU.bypass,
                    )
                    nc.vector.tensor_sub(
                        ptr_row[0:1, 128:256], ptr_row[0:1, 128:256],
                        ptr_row[0:1, 0:128],
                    )
                    offs_ps = ps_small.tile([128, 1], F32, tag="bc")
                    nc.tensor.matmul(
                        offs_ps[:], lhsT=ptr_row[0:1, 128:256],
                        rhs=ones1x128[0:1, 0:1],
                        start=True, stop=True,
                    )
                    offcol = coltmp[:, 12:13]
                    nc.vector.tensor_copy(offcol, offs_ps[:])
                    incl = c32("incl")
                    nc.vector.tensor_scalar(
                        out=incl[:], in0=iscan[:], scalar1=offcol, scalar2=None,
                        op0=ALU.add,
                    )
                    fill = c32("fill")
                    nc.vector.scalar_tensor_tensor(
                        out=fill[:], in0=incl[:], scalar=needcol, in1=maskeq[:],
                        op0=ALU.is_le, op1=ALU.mult,
                    )
                    sel32 = c32("sel32")
                    nc.vector.tensor_scalar(
                        out=sel32[:], in0=z32[:], scalar1=vB, scalar2=None,
                        op0=ALU.is_gt,
                    )
                    nc.vector.tensor_add(sel32[:], sel32[:], fill[:])
                    nc.sync.dma_start(
                        dbg[1:2, :].rearrange("o (p c) -> (o p) c", p=128), sel32[:]
                    )

                    # -- masked softmax in (128,32) -----------------------
                    src = c32("src")
                    nc.scalar.dma_start(
                        src[:], sd[:].rearrange("o (p c) -> (o p) c", p=128)
                    )
                    s32 = c32("s32")
                    c56 = colp.tile([128, 2], F32, name="c56")
                    pe_bcast_col(strip[0:1, 5:7], c56[:])
                    ln_row_fix(s32, src, c56[:, 0:1], c56[:, 1:2])
                    nc.sync.dma_start(
                        dbg[2:3, :].rearrange("o (p c) -> (o p) c", p=128), s32[:]
                    )
                    masked = c32("masked")
                    nc.vector.tensor_scalar(
                        out=masked[:], in0=sel32[:], scalar1=-1.0, scalar2=BIG,
                        op0=ALU.add, op1=ALU.mult,
                    )
                    nc.vector.tensor_add(masked[:], masked[:], s32[:])
                    nc.vector.reduce_max(
                        out=coltmp[:, 13:14], in_=masked[:], axis=mybir.AxisListType.X
                    )
                    smax = strip[0:1, si + 2 : si + 3]
                    col_reduce(coltmp[:, 13:14], smax, "max")
                    nsmax = strip[0:1, si + 3 : si + 4]
                    nc.vector.tensor_scalar(
                        out=nsmax, in0=smax, scalar1=-1.0, scalar2=None, op0=ALU.mult
                    )
                    nsmaxcol = coltmp[:, 14:15]
                    pe_bcast_col(nsmax, nsmaxcol)
                    e32h = colp.tile([128, 32], F16, name="e32h")
                    zsumcol = colp.tile([128, 1], F32, name="zsumcol")
                    nc.scalar.activation(
                        e32h[:], masked[:], AF.Exp, bias=nsmaxcol,
                        accum_out=zsumcol[:, 0:1],
                    )
                    zsum = strip[0:1, si + 4 : si + 5]
                    col_reduce(zsumcol[:, 0:1], zsum, "sum")
                    rz = strip[0:1, si + 5 : si + 6]
                    nc.vector.reciprocal(out=rz, in_=zsum)
                    # ctx weights w = e * r ; sigma = sum(w * m)
                    w32h = colp.tile([128, 32], F16, name="w32h")
                    nc.vector.tensor_mul(w32h[:], e32h[:], r32[:])
                    sig32 = c32("sig32")
                    nc.vector.scalar_tensor_tensor(
                        out=sig32[:], in0=m32[:], scalar=1.0, in1=w32h[:],
                        op0=ALU.mult, op1=ALU.mult, accum_out=coltmp[:, 15:16],
                    )
                    sig = strip[0:1, si + 6 : si + 7]
                    col_reduce(coltmp[:, 15:16], sig, "sum")
                    nc.sync.dma_start(
                        wd[0:1, 0:T].rearrange("o (p c) -> (o p) c", p=128), w32h[:]
                    )

                    # broadcast w row and accumulate ctxraw over yT
                    wB = bigp.tile([128, T], F16, name="wB")
                    for ch in range(NCH):
                        sl = slice(ch * CW, (ch + 1) * CW)
                        wrc = rowp.tile([1, CW], F16, name="wrc", tag="wrc")
                        nc.sync.dma_start(wrc[:], wd[0:1, sl])
                        pb = ps_wide.tile([128, CW], F32, tag="pb")
                        nc.tensor.matmul(
                            pb[:], lhsT=ones1x128h[:], rhs=wrc[:],
                            start=True, stop=True,
                        )
                        if ch % 2 == 0:
                            nc.vector.tensor_copy(wB[:, sl], pb[:])
                        else:
                            nc.scalar.activation(wB[:, sl], pb[:], AF.Copy)
                    scrH = bigp.tile([128, T], F16, name="scrH")
                    for k in range(NK):
                        nc.vector.scalar_tensor_tensor(
                            out=scrH[:],
                            in0=yT[k][:],
                            scalar=1.0,
                            in1=wB[:],
                            op0=ALU.mult,
                            op1=ALU.mult,
                            accum_out=ctx4[:, k : k + 1],
                        )
                    # ctx = (g*ctxraw - sig*g) * rz + b
                    sigcol = coltmp[:, 13:14]
                    pe_bcast_col(sig, sigcol)
                    rzcol = coltmp[:, 14:15]
                    pe_bcast_col(rz, rzcol)
                    nc.vector.tensor_scalar(
                        out=ctx4[:], in0=ctx4[:], scalar1=sigcol, scalar2=None,
                        op0=ALU.subtract,
                    )
                    nc.vector.tensor_mul(ctx4[:], ctx4[:], gcol[:])
                    nc.vector.tensor_scalar(
                        out=ctx4[:], in0=ctx4[:], scalar1=rzcol, scalar2=None,
                        op0=ALU.mult,
                    )
                    nc.vector.tensor_add(ctx4[:], ctx4[:], bcol[:])
                # yT/phase-B pools closed

            # ---------------- allgather + output projection ----------------
            nc.sync.dma_start(cc_in[:], ctx4[:])
            nc.gpsimd.collective_compute(
                "AllGather",
                ALU.bypass,
                replica_groups=[list(range(NCORES))],
                ins=[cc_in[:]],
                outs=[cc_out[:]],
            )
            nc.sync.dma_start(
                ctxall[:].rearrange("p (j b) -> p j b", j=NK),
                cc_out[:].rearrange("(b p) j -> p j b", p=128),
            )
            ctxall_h = cpool.tile([128, 32], F16, name="ctxall_h")
            nc.vector.tensor_copy(ctxall_h[:], ctxall[:])
            nchunks = (VS + CW - 1) // CW
            with (
                tc.tile_pool(name="lo", bufs=4) as lopool,
                tc.tile_pool(name="psl", bufs=4, space="PSUM") as psl,
            ):
                for n in range(nchunks):
                    w = min(CW, VS - n * CW)
                    vsl = slice(n * CW, n * CW + w)
                    bt = lopool.tile([1, CW], F16, name="bo", tag="bo")
                    nc.sync.dma_start(bt[:, :w], bout[:, vsl])
                    pl = psl.tile([B, CW], F32, tag="pl")
                    for k in range(NK):
                        nc.tensor.matmul(
                            pl[:, :w],
                            lhsT=ctxall_h[:, 8 * k : 8 * (k + 1)],
                            rhs=wot[k][:, vsl],
                            start=(k == 0),
                            stop=False,
                        )
                    nc.tensor.matmul(
                        pl[:, :w], lhsT=ones1x8h[:], rhs=bt[:, :w],
                        start=False, stop=True,
                    )
                    lt = lopool.tile([B, CW], F32, name="lt", tag="lt")
                    nc.vector.tensor_copy(lt[:, :w], pl[:, :w])
                    nc.sync.dma_start(logits[:, vsl], lt[:, :w])

    return nc


def _host_prep(inputs):
    f32 = lambda a: np.ascontiguousarray(np.asarray(a, dtype=np.float32))
    f16c = lambda a: np.ascontiguousarray(np.asarray(a, dtype=np.float16))
    seq = np.asarray(inputs["seq"])
    embed = f32(inputs["embed"])
    w1 = f32(inputs["W1"])
    b1 = f32(inputs["b1"])
    w2 = f32(inputs["W2"])
    b2 = f32(inputs["b2"])
    ln_g = f32(inputs["ln_g"])
    ln_b = f32(inputs["ln_b"])
    gw = f32(inputs["gate_W"])
    gb = f32(inputs["gate_b"])
    qw = f32(inputs["q_W"])
    qb = f32(inputs["q_b"])
    wout_f = f32(inputs["out_W"])
    bout_f = f32(inputs["out_b"])

    colpack = lambda v: np.ascontiguousarray(
        v.reshape(-1, 128).T.astype(np.float32)
    )  # (Ntiles*128,) -> (128, Ntiles); tile k col = dims [128k, 128k+128)
    cnt = np.minimum(K, T - 1 - np.arange(T)).astype(np.float32)
    cntrec = np.zeros(T, dtype=np.float32)
    cntrec[cnt > 0] = 1.0 / cnt[cnt > 0]

    gw1 = gw[:D, 0]
    gw2 = gw[D:, 0]
    gw1g = (gw1 * ln_g).astype(np.float32)
    gw2g = (gw2 * ln_g).astype(np.float32)
    # rows3: per k-tile, cols [ones, gw1g_k, gw2g_k]
    g1p, g2p = colpack(gw1g), colpack(gw2g)
    rows3 = np.zeros((128, 3 * NK), dtype=np.float16)
    for k in range(NK):
        rows3[:, 3 * k] = 1.0
        rows3[:, 3 * k + 1] = g1p[:, k]
        rows3[:, 3 * k + 2] = g2p[:, k]
    c1 = float(np.dot(gw1, ln_g))
    c2 = float(np.dot(gw1, ln_b))
    c3 = float(np.dot(gw2, ln_g))
    c4 = float(np.dot(gw2, ln_b))

    base = {
        "embed_h": f16c(embed),
        "w1h": f16c(w1),
        "w2h": f16c(w2),
        "qwh": f16c(qw),
        "b1c": colpack(b1),
        "b2c": colpack(b2),
        "gcol": colpack(ln_g),
        "bcol": colpack(ln_b),
        "rows3": np.ascontiguousarray(rows3),
        "qbc": colpack(qb),
        "ident": np.eye(128, dtype=np.float32),
        "onesc": np.ones((128, 1), dtype=np.float32),
        "ones1x128": np.ones((1, 128), dtype=np.float32),
        "ones1x128h": np.ones((1, 128), dtype=np.float16),
        "ones1x8h": np.ones((1, 8), dtype=np.float16),
        "alphac": ((np.arange(128, dtype=np.float32) + 1.0) / 128.0).reshape(128, 1),
        "cnt32": np.ascontiguousarray(cntrec.reshape(128, 32)),
        "sc_in": np.array(
            [[EPS, float(gb[0]), c1, c2, c3, c4, 0.0, 0.0]], dtype=np.float32
        ),
    }
    wout_pad = np.zeros((D, NCORES * VS), dtype=np.float32)
    wout_pad[:, :V] = wout_f
    bout_pad = np.zeros(NCORES * VS, dtype=np.float32)
    bout_pad[:V] = bout_f

    in_maps = []
    for c in range(NCORES):
        m = dict(base)
        m["seq_idx"] = np.ascontiguousarray(
            seq[c].reshape(32, 128).T.astype(np.int32)
        )
        m["wout"] = np.ascontiguousarray(
            wout_pad[:, c * VS : (c + 1) * VS].astype(np.float16)
        )
        m["bout"] = np.ascontiguousarray(
            bout_pad[c * VS : (c + 1) * VS].reshape(1, VS).astype(np.float16)
        )
        in_maps.append(m)
    return in_maps


def get_nc():
    key = (DEBUG_HT,)
    if key not in _cache:
        _cache[key] = build_bass()
    return _cache[key]


def run_full(inputs, trace=False):
    """Run the kernel; returns (logits_full, BassKernelResults)."""
    nc = get_nc()
    in_maps = _host_prep(inputs)
    res = run_bass_kernel_spmd(
        nc, in_maps, core_ids=list(range(NCORES)), trace=trace
    )
    parts = [res.results[c]["logits"] for c in range(NCORES)]
    logits = np.concatenate(parts, axis=1)[:, :V]
    return logits, res


def kernel(**inputs) -> np.ndarray:
    logits, _ = run_full(inputs, trace=False)
    return logits
